# revision 56
# baseline (speedup 1.0000x reference)
"""Multi-head attention (B=2, S=2048, D=1024, H=16) on 8 Trainium2 cores.

Sharding: head-parallel. Core c owns heads {2c, 2c+1} (a contiguous
128-wide slice of the projection space). Each core reads the full
(transposed, bf16) activations, computes its heads' Q/K/V projections,
full S x S attention, and its partial contribution to the output
projection (row-parallel Wo). Host sums the 8 partials.

Device-side layout notes:
  - Scores are computed transposed (scoresT[k, q]) so the softmax
    contraction (over k) lands on the PSUM partition axis, where the
    tensor engine can both re-sum it and contract it with V.
  - Attention runs one head at a time over 1024-wide q tiles, one
    matmul per 128-key chunk (N=1024 spanning two PSUM banks), so each
    chunk parks at most one attn@V matmul in the PE wait queue and the
    exp stream on the scalar engine never stalls on sequencer backup.
  - V carries an extra all-ones 65th column, so the attn@V matmul
    (M=65) also accumulates the softmax denominator Z into PSUM row 64
    for free -- no separate denominator matmuls on the PE.
  - The key-padding mask is a per-partition (per-key) bias of -30000
    applied inside the exp activation (out = exp(in*scale + bias)), so
    masked keys' attention weights are exactly 0 at zero extra cost.
  - V is projected directly in [key-row, dim] orientation (no DMA
    transposes), copied into a per-head [V | ones] SBUF layout.
  - 1/Z (one partition row) is broadcast across the head's 64
    partitions by K=1 bf16 ones-matmuls, staged through SBUF (DVE
    reads at most one PSUM operand); tensor_muls write the normalized
    O^T half into the stacked [128, S] otn used as lhsT by the K=128
    output projection. Head 1's half reaches partitions 64:128 via a
    partition-shifting DMA copy (engines cannot shift partitions).
  - Software pipelining against the in-order PE queue: the tail of
    batch 0's projections, all of batch 1's projections, and both
    output projections are emitted as filler inside the (exp-paced)
    attention chunk loops, so the PE stays busy while the scalar
    engine works through the exps. Pass-i's normalization is emitted
    inside pass i+1 so the reciprocal latency never stalls the exps.
  - Output partials are stored fp16; the host accumulates in fp32.
    Stores ride the GPSIMD (Pool) SWDGE queue so they never
    head-of-line block the activation loads on the SP queue.
"""

import math

import ml_dtypes
import numpy as np

B, S, D, H = 2, 2048, 1024, 16
DH = D // H            # 64
NCORES = 8
MH = 2 * DH            # 128: per-core slice of the head dim (2 heads)
BS = B * S             # 4096
PK = S // 128          # 16 key chunks per batch
PD = D // 128          # 8 contraction chunks for the projections
QT = 512               # projection q-tile width
QH = 512               # attention q half-tile (PSUM bank width)
VW = DH + 1            # 65: V columns incl. the ones column
SCALE = 1.0 / math.sqrt(DH)
BF16 = ml_dtypes.bfloat16

_NC_CACHE = {}


class _Filler:
    """FIFO of generators that emit PE-filler instructions on demand."""

    def __init__(self, *gens):
        self.gens = list(gens)

    def add(self, gen):
        self.gens.append(gen)

    def take(self, n):
        while n > 0 and self.gens:
            try:
                next(self.gens[0])
                n -= 1
            except StopIteration:
                self.gens.pop(0)


def _build_nc():
    """Build the (core-independent) Bass program once."""
    if "nc" in _NC_CACHE:
        return _NC_CACHE["nc"]

    from contextlib import ExitStack

    import concourse.bacc as bacc
    import concourse.mybir as mybir
    import concourse.tile as tile

    f32 = mybir.dt.float32
    f32r = mybir.dt.float32r
    f16 = mybir.dt.float16
    bf16 = mybir.dt.bfloat16
    Exp = mybir.ActivationFunctionType.Exp

    nc = bacc.Bacc("TRN2", target_bir_lowering=False, debug=False)

    xqT = nc.dram_tensor("xqT", [D, BS], bf16, kind="ExternalInput").ap()
    xkT = nc.dram_tensor("xkT", [D, BS], bf16, kind="ExternalInput").ap()
    xvT = nc.dram_tensor("xvT", [D, BS], bf16, kind="ExternalInput").ap()
    wq = nc.dram_tensor("wq", [128, PD, MH], bf16, kind="ExternalInput").ap()
    wk = nc.dram_tensor("wk", [128, PD, MH], bf16, kind="ExternalInput").ap()
    wv = nc.dram_tensor("wv", [128, PD, MH], bf16, kind="ExternalInput").ap()
    wo = nc.dram_tensor("wo", [128, D], bf16, kind="ExternalInput").ap()
    mb = nc.dram_tensor("mb", [128, B, PK], f32, kind="ExternalInput").ap()
    out = nc.dram_tensor("out", [BS, D], f16, kind="ExternalOutput").ap()

    with tile.TileContext(nc) as tc, ExitStack() as ctx:
        wpool = ctx.enter_context(tc.tile_pool(name="wpool", bufs=1))
        apool = ctx.enter_context(tc.tile_pool(name="apool", bufs=1))

        wq_sb = wpool.tile([128, PD, MH], bf16)
        wk_sb = wpool.tile([128, PD, MH], bf16)
        wv_sb = wpool.tile([128, PD, MH], bf16)
        wo_sb = wpool.tile([128, D], bf16)
        mb_sb = wpool.tile([128, B, PK], f32)
        onesb_sb = wpool.tile([128, DH], bf16)
        nc.vector.memset(onesb_sb, 1.0)

        # Persistent per-core activations:
        #   qT_sb/kT_sb/vT_sb: [128 = 2 heads x 64 head-dims, BS] transposed
        #   v2_sb: [128 key positions, b, chunk, head, 64 dims + ones col]
        qT_sb = apool.tile([128, BS], bf16)
        kT_sb = apool.tile([128, BS], bf16)
        v2_sb = apool.tile([128, B, PK, 2, VW], bf16)
        nc.vector.memset(v2_sb[:, :, :, :, DH:DH + 1], 1.0)

        xhp = ctx.enter_context(tc.tile_pool(name="xhp", bufs=3))
        with (
            tc.tile_pool(name="atp", bufs=8) as atp,
            tc.tile_pool(name="rzp", bufs=2) as rzp,
            tc.tile_pool(name="op", bufs=2) as op,
            tc.tile_pool(name="outp", bufs=4) as outp,
            tc.tile_pool(name="psp", bufs=2, space="PSUM") as psp,
            tc.tile_pool(name="pss", bufs=2, space="PSUM") as pss,
            tc.tile_pool(name="pso", bufs=1, space="PSUM") as pso,
        ):
            TENSORS = {"q": (xqT, wq_sb, qT_sb), "k": (xkT, wk_sb, kT_sb),
                       "v": (xvT, wv_sb, None)}

            def emit_load(t, b, xh, cs0, cw):
                xT = TENSORS[t][0]
                src = xT[:, b * S + cs0:b * S + cs0 + cw]
                nc.sync.dma_start(
                    xh[:, :, cs0:cs0 + cw],
                    src.rearrange("(kc p) c -> p kc c", p=128),
                )

            def emit_group(t, b, xh, sti):
                if t == "v":
                    # V is projected directly in [key-row, dim] orientation
                    # (no transposes): per 128-key chunk, 8 accumulating
                    # N=128 matmuls, then a strided copy into v2.
                    for kci in range(sti * 4, sti * 4 + 4):
                        ks = kci * 128
                        pv = psp.tile([128, QT], f32, tag="pq", name="pv")
                        for cc in range(PD):
                            nc.tensor.matmul(
                                pv[:, 0:128],
                                lhsT=xh[:, cc, ks:ks + 128],
                                rhs=wv_sb[:, cc, :],
                                start=(cc == 0),
                                stop=(cc == PD - 1),
                            )
                            if cc % 4 == 3:
                                yield
                        nc.vector.tensor_copy(
                            v2_sb[:, b, kci, :, 0:DH], pv[:, 0:128]
                        )
                    return
                w_sb, dst = TENSORS[t][1], TENSORS[t][2]
                pq = psp.tile([128, QT], f32, tag="pq", name="pq")
                for kc in range(PD):
                    nc.tensor.matmul(
                        pq,
                        lhsT=w_sb[:, kc, :],
                        rhs=xh[:, kc, sti * QT:(sti + 1) * QT],
                        start=(kc == 0),
                        stop=(kc == PD - 1),
                    )
                    yield
                ds = b * S + sti * QT
                nc.vector.tensor_copy(dst[:, ds:ds + QT], pq)

            def run_gen(g):
                for _ in g:
                    pass

            def gen_proj(b):
                for t in ("q", "k", "v"):
                    xh = xhp.tile([128, PD, S], bf16, tag="xh", name="xh")
                    emit_load(t, b, xh, 0, 1024)
                    emit_load(t, b, xh, 1024, 1024)
                    for sti in range(S // QT):
                        yield from emit_group(t, b, xh, sti)

            def gen_outproj(b, otn, st_lo, st_hi, tail=False):
                # In the tail (no exps left) the score PSUM ring is free:
                # use its wider slots and split the copies across DVE and
                # ACT so the psum-ring turnaround never paces the drain.
                for st in range(st_lo, st_hi):
                    rs = b * S + st * 128
                    ws = outp.tile([128, D], f16, tag="ws", name="ws")
                    if tail:
                        if st % 2 == 0:
                            wp2 = pss.tile([128, 2, QH], f32, tag="sc",
                                           name="wp2")
                            for nt in range(2):
                                nc.tensor.matmul(
                                    wp2[:, nt, :],
                                    lhsT=otn[:, st * 128:(st + 1) * 128],
                                    rhs=wo_sb[:, nt * QH:(nt + 1) * QH],
                                )
                                yield
                            nc.vector.tensor_copy(ws[:, 0:QH], wp2[:, 0, :])
                            nc.scalar.copy(ws[:, QH:D], wp2[:, 1, :])
                            # Alternate store queues so the serial SWDGE
                            # generation never paces the drain.
                            nc.sync.dma_start(out[rs:rs + 128, :], ws)
                            continue
                        for nt in range(2):
                            wp = psp.tile([128, QH], f32, tag="pq",
                                          name="wp")
                            nc.tensor.matmul(
                                wp,
                                lhsT=otn[:, st * 128:(st + 1) * 128],
                                rhs=wo_sb[:, nt * QH:(nt + 1) * QH],
                            )
                            yield
                            if nt == 0:
                                nc.vector.tensor_copy(ws[:, 0:QH], wp)
                            else:
                                nc.scalar.copy(ws[:, QH:D], wp)
                    else:
                        for nt in range(2):
                            wp = psp.tile([128, QH], f32, tag="pq",
                                          name="wp")
                            nc.tensor.matmul(
                                wp,
                                lhsT=otn[:, st * 128:(st + 1) * 128],
                                rhs=wo_sb[:, nt * QH:(nt + 1) * QH],
                            )
                            yield
                            nc.vector.tensor_copy(
                                ws[:, nt * QH:(nt + 1) * QH], wp
                            )
                    nc.gpsimd.dma_start(out[rs:rs + 128, :], ws)

            def make_norm(h, qt, ot, otn):
                # 1/Z on the single Z partition row, broadcast across the
                # head's 64 partitions via K=1 fp32r matmuls, staged
                # through SBUF (DVE ops keep one PSUM operand max), then
                # the normalized O^T half lands in otn -- directly for
                # head 0; via a partition-shifting DMA copy for head 1
                # (engines cannot shift partitions; the DMA can).
                # Emitted deferred -- inside the NEXT pass, after its
                # first two score matmuls -- so the reciprocal latency
                # never delays the exp stream.
                def emit(between=None):
                    rz = rzp.tile([128, 2, QH], bf16, tag="rz", name="rz")
                    rbs = rzp.tile([128, 2, QH], f32, tag="rbs", name="rbs")
                    dst = otn if h == 0 else op.tile(
                        [128, 1024], bf16, tag="ott", name="ott", bufs=2
                    )
                    dc = qt * 1024 if h == 0 else 0
                    with nc.allow_low_precision("1/Z broadcast in bf16"):
                        nc.vector.reciprocal(
                            rz[DH:DH + 1, :, :], ot[DH:DH + 1, :, :]
                        )
                    for qh in range(2):
                        rb = psp.tile([128, QH], f32, tag="pq", name="rb")
                        nc.tensor.matmul(
                            rb[0:DH, :],
                            lhsT=onesb_sb[DH:DH + 1, 0:DH],
                            rhs=rz[DH:DH + 1, qh, :],
                        )
                        # Stage 1/Z through SBUF: a DVE tensor op may read
                        # at most one PSUM operand.
                        nc.vector.tensor_copy(rbs[0:DH, qh, :], rb[0:DH, :])
                        nc.vector.tensor_mul(
                            dst[0:DH, dc + qh * QH:dc + (qh + 1) * QH],
                            ot[0:DH, qh, :], rbs[0:DH, qh, :],
                        )
                        if qh == 0 and between is not None:
                            between()
                    if h == 1:
                        nc.sync.dma_start(
                            otn[DH:128, qt * 1024:(qt + 1) * 1024],
                            dst[0:DH, :],
                        )
                return emit

            def attn_pass(b, h, qt, otn, filler, norm_prev=None, takes=None):
                hs = h * DH
                qs = b * S + qt * 1024

                def sc_mm(kc):
                    ks = b * S + kc * 128
                    sc = pss.tile([128, 2, QH], f32, tag="sc", name="sc")
                    for qh in range(2):
                        nc.tensor.matmul(
                            sc[:, qh, :],
                            lhsT=kT_sb[hs:hs + DH, ks:ks + 128],
                            rhs=qT_sb[hs:hs + DH,
                                      qs + qh * QH:qs + (qh + 1) * QH],
                        )
                    return sc

                ot = pso.tile([128, 2, QH], f32, tag="ot", name="ot")
                sc = sc_mm(0)
                for kc in range(PK):
                    attn = atp.tile([128, 2, QH], bf16, tag="attn",
                                    name="attn")
                    nc.scalar.activation(attn, sc, Exp, scale=SCALE,
                                         bias=mb_sb[:, b, kc:kc + 1])
                    if kc < PK - 1:
                        sc = sc_mm(kc + 1)
                    if kc == 0 and norm_prev is not None:
                        norm_prev()
                    # O^T[d, q] += V[k, d]^T attn[k, q]; row 64 (the ones
                    # column) accumulates Z = sum_k attn[k, q].
                    for qh in range(2):
                        nc.tensor.matmul(
                            ot[0:VW, qh, :],
                            lhsT=v2_sb[:, b, kc, h, :],
                            rhs=attn[:, qh, :],
                            start=(kc == 0), stop=(kc == PK - 1),
                            skip_group_check=True,
                        )
                    filler.take(takes.get(kc, 1) if takes else 1)
                    if kc == 0 and norm_prev is not None:
                        filler.take(4)
                return make_norm(h, qt, ot, otn)

            # ---- batch 0 prologue -------------------------------------
            # Serial prefix = only what the first attention chunks need:
            # K slice 0, V chunks 0-3, Q slices 0-1. Everything else is
            # filler inside pass 1, paced to the staged DMA arrivals.
            nc.sync.dma_start(wk_sb, wk)
            xhk = xhp.tile([128, PD, S], bf16, tag="xh", name="xhk")
            xhv = xhp.tile([128, PD, S], bf16, tag="xh", name="xhv")
            xhq = xhp.tile([128, PD, S], bf16, tag="xh", name="xhq")
            emit_load("k", 0, xhk, 0, 512)
            nc.sync.dma_start(wq_sb, wq)
            nc.sync.dma_start(wv_sb, wv)
            nc.sync.dma_start(wo_sb, wo)
            nc.sync.dma_start(mb_sb, mb)
            # Tiny warm-up ops: let DVE/ACT observe the mask DMA early and
            # pull the ~2.7us exp table load off the critical path.
            scratch = wpool.tile([1, 2], f32)
            nc.vector.tensor_copy(scratch, mb_sb[0:1, 0, 0:2])
            scratch2 = wpool.tile([1, 2], f32)
            nc.scalar.activation(scratch2, mb_sb[0:1, 0, 0:2], Exp)
            emit_load("v", 0, xhv, 0, 512)
            emit_load("q", 0, xhq, 0, 512)
            emit_load("q", 0, xhq, 512, 512)
            emit_load("k", 0, xhk, 512, 512)
            emit_load("v", 0, xhv, 512, 512)
            emit_load("k", 0, xhk, 1024, 512)
            emit_load("v", 0, xhv, 1024, 512)
            emit_load("k", 0, xhk, 1536, 512)
            emit_load("v", 0, xhv, 1536, 512)
            emit_load("q", 0, xhq, 1024, 512)
            emit_load("q", 0, xhq, 1536, 512)

            run_gen(emit_group("k", 0, xhk, 0))
            run_gen(emit_group("v", 0, xhv, 0))
            run_gen(emit_group("q", 0, xhq, 0))
            run_gen(emit_group("q", 0, xhq, 1))

            def gen_b0_rest():
                yield from emit_group("k", 0, xhk, 1)
                yield from emit_group("v", 0, xhv, 1)
                yield from emit_group("k", 0, xhk, 2)
                yield from emit_group("v", 0, xhv, 2)
                yield from emit_group("k", 0, xhk, 3)
                yield from emit_group("v", 0, xhv, 3)
                yield from emit_group("q", 0, xhq, 2)
                yield from emit_group("q", 0, xhq, 3)

            # ---- attention, software-pipelined ------------------------
            fil = _Filler(gen_b0_rest(), gen_proj(1))
            otn0 = op.tile([128, S], bf16, tag="otn", name="otn0")
            otn1 = op.tile([128, S], bf16, tag="otn", name="otn1")
            # pass 1: K/V/Q tails land mid-pass; pace the filler to the
            # staged DMA arrivals.
            nrm = attn_pass(0, 0, 0, otn0, fil,
                            takes={0: 8, 1: 8, 2: 0, 3: 0, 4: 8, 5: 0,
                                   6: 8, 7: 0, 8: 8, 9: 0, 10: 8, 11: 0,
                                   12: 8, 13: 0, 14: 8, 15: 0})
            nrm = attn_pass(0, 0, 1, otn0, fil, norm_prev=nrm,
                            takes={kc: 2 if kc % 2 == 0 else 1
                                   for kc in range(PK)})
            nrm = attn_pass(0, 1, 0, otn0, fil, norm_prev=nrm,
                            takes={kc: 2 if kc % 2 == 0 else 1
                                   for kc in range(PK)})
            nrm = attn_pass(0, 1, 1, otn0, fil, norm_prev=nrm,
                            takes={kc: 2 for kc in range(PK)})

            # Batch 1 runs head 1 first so the LAST pass is head 0, whose
            # normalization writes otn directly -- no partition-shift DMA
            # on the tail critical path.
            fil.add(gen_outproj(0, otn0, 0, S // 128))
            nrm = attn_pass(1, 1, 0, otn1, fil, norm_prev=nrm,
                            takes={kc: 2 if kc < 8 else 1
                                   for kc in range(PK)})
            nrm = attn_pass(1, 1, 1, otn1, fil, norm_prev=nrm)
            nrm = attn_pass(1, 0, 0, otn1, fil, norm_prev=nrm)
            fil.add(gen_outproj(1, otn1, 0, 8))
            nrm = attn_pass(1, 0, 1, otn1, fil, norm_prev=nrm)

            # ---- tail: final normalization + remaining output proj ----
            # The first half of the remaining output projection depends
            # only on the qh=0 normalization half; interleave it so it
            # overlaps the qh=1 chain.
            nrm(between=lambda: run_gen(gen_outproj(1, otn1, 8, 12,
                                                    tail=True)))
            fil.add(gen_outproj(1, otn1, 12, S // 128, tail=True))
            fil.take(1 << 20)

    nc.compile()
    _NC_CACHE["nc"] = nc
    return nc


def _prep_inputs(queries, keys, values, masks, Wq, Wk, Wv, Wo):
    """Host-side sharding/layout prep. Returns per-core input maps."""
    def t_bf16(x):  # [B, S, D] -> [D, B*S] bf16, contiguous
        return np.ascontiguousarray(
            np.asarray(x, dtype=np.float32).reshape(BS, D).astype(BF16).T
        )

    xqT, xkT, xvT = t_bf16(queries), t_bf16(keys), t_bf16(values)

    m01 = (np.asarray(masks) != 0).astype(np.float32)          # [B, S]
    mb = np.ascontiguousarray(
        np.where(m01.reshape(B, PK, 128) != 0, 0.0, -30000.0)
        .transpose(2, 0, 1).astype(np.float32)
    )

    def w_prep(W, c):  # [D, D] -> [128, PD, MH] bf16 slice for core c
        Wc = np.asarray(W, dtype=np.float32)[:, c * MH:(c + 1) * MH]
        return np.ascontiguousarray(
            Wc.astype(BF16).reshape(PD, 128, MH).transpose(1, 0, 2)
        )

    Wo_f = np.asarray(Wo, dtype=np.float32)
    in_maps = []
    for c in range(NCORES):
        in_maps.append({
            "xqT": xqT, "xkT": xkT, "xvT": xvT,
            "wq": w_prep(Wq, c), "wk": w_prep(Wk, c), "wv": w_prep(Wv, c),
            "wo": np.ascontiguousarray(
                Wo_f[c * MH:(c + 1) * MH, :].astype(BF16)
            ),
            "mb": mb,
        })
    return in_maps


def run(inputs, trace=False, trace_cores=None):
    """Run on 8 NeuronCores; returns (output [B,S,D] f32, BassKernelResults)."""
    from concourse.bass_utils import run_bass_kernel_spmd

    nc = _build_nc()
    in_maps = _prep_inputs(**inputs)
    res = run_bass_kernel_spmd(
        nc, in_maps, core_ids=list(range(NCORES)),
        trace=trace, trace_cores=trace_cores,
    )
    acc = res.results[0]["out"].astype(np.float32, copy=True)
    for r in res.results[1:]:
        acc += r["out"]
    return acc.reshape(B, S, D), res


def kernel(**inputs) -> np.ndarray:
    out, _ = run(inputs)
    return out


# revision 57
# speedup vs baseline: 1.0018x; 1.0018x over previous
"""Multi-head attention (B=2, S=2048, D=1024, H=16) on 8 Trainium2 cores.

Sharding: head-parallel. Core c owns heads {2c, 2c+1} (a contiguous
128-wide slice of the projection space). Each core reads the full
(transposed, bf16) activations, computes its heads' Q/K/V projections,
full S x S attention, and its partial contribution to the output
projection (row-parallel Wo). Host sums the 8 partials.

Device-side layout notes:
  - Scores are computed transposed (scoresT[k, q]) so the softmax
    contraction (over k) lands on the PSUM partition axis, where the
    tensor engine can both re-sum it and contract it with V.
  - Attention runs one head at a time over 1024-wide q tiles, one
    matmul per 128-key chunk (N=1024 spanning two PSUM banks), so each
    chunk parks at most one attn@V matmul in the PE wait queue and the
    exp stream on the scalar engine never stalls on sequencer backup.
  - V carries an extra all-ones 65th column, so the attn@V matmul
    (M=65) also accumulates the softmax denominator Z into PSUM row 64
    for free -- no separate denominator matmuls on the PE.
  - The key-padding mask is a per-partition (per-key) bias of -30000
    applied inside the exp activation (out = exp(in*scale + bias)), so
    masked keys' attention weights are exactly 0 at zero extra cost.
  - V is projected directly in [key-row, dim] orientation (no DMA
    transposes), copied into a per-head [V | ones] SBUF layout.
  - 1/Z (one partition row) is broadcast across the head's 64
    partitions by K=1 bf16 ones-matmuls, staged through SBUF (DVE
    reads at most one PSUM operand); tensor_muls write the normalized
    O^T half into the stacked [128, S] otn used as lhsT by the K=128
    output projection. Head 1's half reaches partitions 64:128 via a
    partition-shifting DMA copy (engines cannot shift partitions).
  - Software pipelining against the in-order PE queue: the tail of
    batch 0's projections, all of batch 1's projections, and both
    output projections are emitted as filler inside the (exp-paced)
    attention chunk loops, so the PE stays busy while the scalar
    engine works through the exps. Pass-i's normalization is emitted
    inside pass i+1 so the reciprocal latency never stalls the exps.
  - Output partials are stored fp16; the host accumulates in fp32.
    Stores ride the GPSIMD (Pool) SWDGE queue so they never
    head-of-line block the activation loads on the SP queue.
"""

import math

import ml_dtypes
import numpy as np

B, S, D, H = 2, 2048, 1024, 16
DH = D // H            # 64
NCORES = 8
MH = 2 * DH            # 128: per-core slice of the head dim (2 heads)
BS = B * S             # 4096
PK = S // 128          # 16 key chunks per batch
PD = D // 128          # 8 contraction chunks for the projections
QT = 512               # projection q-tile width
QH = 512               # attention q half-tile (PSUM bank width)
VW = DH + 1            # 65: V columns incl. the ones column
SCALE = 1.0 / math.sqrt(DH)
BF16 = ml_dtypes.bfloat16

_NC_CACHE = {}


class _Filler:
    """FIFO of generators that emit PE-filler instructions on demand."""

    def __init__(self, *gens):
        self.gens = list(gens)

    def add(self, gen):
        self.gens.append(gen)

    def take(self, n):
        while n > 0 and self.gens:
            try:
                next(self.gens[0])
                n -= 1
            except StopIteration:
                self.gens.pop(0)


def _build_nc():
    """Build the (core-independent) Bass program once."""
    if "nc" in _NC_CACHE:
        return _NC_CACHE["nc"]

    from contextlib import ExitStack

    import concourse.bacc as bacc
    import concourse.mybir as mybir
    import concourse.tile as tile

    f32 = mybir.dt.float32
    f32r = mybir.dt.float32r
    f16 = mybir.dt.float16
    bf16 = mybir.dt.bfloat16
    Exp = mybir.ActivationFunctionType.Exp

    nc = bacc.Bacc("TRN2", target_bir_lowering=False, debug=False)

    xqT = nc.dram_tensor("xqT", [D, BS], bf16, kind="ExternalInput").ap()
    xkT = nc.dram_tensor("xkT", [D, BS], bf16, kind="ExternalInput").ap()
    xvT = nc.dram_tensor("xvT", [D, BS], bf16, kind="ExternalInput").ap()
    wq = nc.dram_tensor("wq", [128, PD, MH], bf16, kind="ExternalInput").ap()
    wk = nc.dram_tensor("wk", [128, PD, MH], bf16, kind="ExternalInput").ap()
    wv = nc.dram_tensor("wv", [128, PD, MH], bf16, kind="ExternalInput").ap()
    wo = nc.dram_tensor("wo", [128, D], bf16, kind="ExternalInput").ap()
    mb = nc.dram_tensor("mb", [128, B, PK], f32, kind="ExternalInput").ap()
    out = nc.dram_tensor("out", [BS, D], f16, kind="ExternalOutput").ap()

    with tile.TileContext(nc) as tc, ExitStack() as ctx:
        wpool = ctx.enter_context(tc.tile_pool(name="wpool", bufs=1))
        apool = ctx.enter_context(tc.tile_pool(name="apool", bufs=1))

        wq_sb = wpool.tile([128, PD, MH], bf16)
        wk_sb = wpool.tile([128, PD, MH], bf16)
        wv_sb = wpool.tile([128, PD, MH], bf16)
        wo_sb = wpool.tile([128, D], bf16)
        mb_sb = wpool.tile([128, B, PK], f32)
        onesb_sb = wpool.tile([128, DH], bf16)
        nc.vector.memset(onesb_sb, 1.0)

        # Persistent per-core activations:
        #   qT_sb/kT_sb/vT_sb: [128 = 2 heads x 64 head-dims, BS] transposed
        #   v2_sb: [128 key positions, b, chunk, head, 64 dims + ones col]
        qT_sb = apool.tile([128, BS], bf16)
        kT_sb = apool.tile([128, BS], bf16)
        v2_sb = apool.tile([128, B, PK, 2, VW], bf16)
        nc.vector.memset(v2_sb[:, :, :, :, DH:DH + 1], 1.0)

        xhp = ctx.enter_context(tc.tile_pool(name="xhp", bufs=3))
        with (
            tc.tile_pool(name="atp", bufs=8) as atp,
            tc.tile_pool(name="rzp", bufs=2) as rzp,
            tc.tile_pool(name="op", bufs=2) as op,
            tc.tile_pool(name="outp", bufs=4) as outp,
            tc.tile_pool(name="psp", bufs=2, space="PSUM") as psp,
            tc.tile_pool(name="pss", bufs=2, space="PSUM") as pss,
            tc.tile_pool(name="pso", bufs=1, space="PSUM") as pso,
        ):
            TENSORS = {"q": (xqT, wq_sb, qT_sb), "k": (xkT, wk_sb, kT_sb),
                       "v": (xvT, wv_sb, None)}

            def emit_load(t, b, xh, cs0, cw):
                xT = TENSORS[t][0]
                src = xT[:, b * S + cs0:b * S + cs0 + cw]
                nc.sync.dma_start(
                    xh[:, :, cs0:cs0 + cw],
                    src.rearrange("(kc p) c -> p kc c", p=128),
                )

            def emit_group(t, b, xh, sti):
                if t == "v":
                    # V is projected directly in [key-row, dim] orientation
                    # (no transposes): per 128-key chunk, 8 accumulating
                    # N=128 matmuls, then a strided copy into v2.
                    for kci in range(sti * 4, sti * 4 + 4):
                        ks = kci * 128
                        pv = psp.tile([128, QT], f32, tag="pq", name="pv")
                        for cc in range(PD):
                            nc.tensor.matmul(
                                pv[:, 0:128],
                                lhsT=xh[:, cc, ks:ks + 128],
                                rhs=wv_sb[:, cc, :],
                                start=(cc == 0),
                                stop=(cc == PD - 1),
                            )
                            if cc % 4 == 3:
                                yield
                        nc.vector.tensor_copy(
                            v2_sb[:, b, kci, :, 0:DH], pv[:, 0:128]
                        )
                    return
                w_sb, dst = TENSORS[t][1], TENSORS[t][2]
                pq = psp.tile([128, QT], f32, tag="pq", name="pq")
                for kc in range(PD):
                    nc.tensor.matmul(
                        pq,
                        lhsT=w_sb[:, kc, :],
                        rhs=xh[:, kc, sti * QT:(sti + 1) * QT],
                        start=(kc == 0),
                        stop=(kc == PD - 1),
                    )
                    yield
                ds = b * S + sti * QT
                nc.vector.tensor_copy(dst[:, ds:ds + QT], pq)

            def run_gen(g):
                for _ in g:
                    pass

            def gen_proj(b):
                for t in ("q", "k", "v"):
                    xh = xhp.tile([128, PD, S], bf16, tag="xh", name="xh")
                    emit_load(t, b, xh, 0, 1024)
                    emit_load(t, b, xh, 1024, 1024)
                    for sti in range(S // QT):
                        yield from emit_group(t, b, xh, sti)

            def gen_outproj(b, otn, st_lo, st_hi, tail=False):
                # In the tail (no exps left) the score PSUM ring is free:
                # use its wider slots and split the copies across DVE and
                # ACT so the psum-ring turnaround never paces the drain.
                for st in range(st_lo, st_hi):
                    rs = b * S + st * 128
                    ws = outp.tile([128, D], f16, tag="ws", name="ws")
                    if tail:
                        if st % 2 == 0:
                            wp2 = pss.tile([128, 2, QH], f32, tag="sc",
                                           name="wp2")
                            for nt in range(2):
                                nc.tensor.matmul(
                                    wp2[:, nt, :],
                                    lhsT=otn[:, st * 128:(st + 1) * 128],
                                    rhs=wo_sb[:, nt * QH:(nt + 1) * QH],
                                )
                                yield
                            nc.vector.tensor_copy(ws[:, 0:QH], wp2[:, 0, :])
                            nc.scalar.copy(ws[:, QH:D], wp2[:, 1, :])
                            # Alternate store queues so the serial SWDGE
                            # generation never paces the drain.
                            nc.sync.dma_start(out[rs:rs + 128, :], ws)
                            continue
                        for nt in range(2):
                            wp = psp.tile([128, QH], f32, tag="pq",
                                          name="wp")
                            nc.tensor.matmul(
                                wp,
                                lhsT=otn[:, st * 128:(st + 1) * 128],
                                rhs=wo_sb[:, nt * QH:(nt + 1) * QH],
                            )
                            yield
                            if nt == 0:
                                nc.vector.tensor_copy(ws[:, 0:QH], wp)
                            else:
                                nc.scalar.copy(ws[:, QH:D], wp)
                    else:
                        for nt in range(2):
                            wp = psp.tile([128, QH], f32, tag="pq",
                                          name="wp")
                            nc.tensor.matmul(
                                wp,
                                lhsT=otn[:, st * 128:(st + 1) * 128],
                                rhs=wo_sb[:, nt * QH:(nt + 1) * QH],
                            )
                            yield
                            nc.vector.tensor_copy(
                                ws[:, nt * QH:(nt + 1) * QH], wp
                            )
                    nc.gpsimd.dma_start(out[rs:rs + 128, :], ws)

            def make_norm(h, qt, ot, otn):
                # 1/Z on the single Z partition row, broadcast across the
                # head's 64 partitions via K=1 fp32r matmuls, staged
                # through SBUF (DVE ops keep one PSUM operand max), then
                # the normalized O^T half lands in otn -- directly for
                # head 0; via a partition-shifting DMA copy for head 1
                # (engines cannot shift partitions; the DMA can).
                # Emitted deferred -- inside the NEXT pass, after its
                # first two score matmuls -- so the reciprocal latency
                # never delays the exp stream.
                def emit(between=None):
                    rz = rzp.tile([128, 2, QH], bf16, tag="rz", name="rz")
                    rbs = rzp.tile([128, 2, QH], f32, tag="rbs", name="rbs")
                    dst = otn if h == 0 else op.tile(
                        [128, 1024], bf16, tag="ott", name="ott", bufs=2
                    )
                    dc = qt * 1024 if h == 0 else 0
                    with nc.allow_low_precision("1/Z broadcast in bf16"):
                        nc.vector.reciprocal(
                            rz[DH:DH + 1, :, :], ot[DH:DH + 1, :, :]
                        )
                    for qh in range(2):
                        rb = psp.tile([128, QH], f32, tag="pq", name="rb")
                        nc.tensor.matmul(
                            rb[0:DH, :],
                            lhsT=onesb_sb[DH:DH + 1, 0:DH],
                            rhs=rz[DH:DH + 1, qh, :],
                        )
                        # Stage 1/Z through SBUF: a DVE tensor op may read
                        # at most one PSUM operand.
                        nc.vector.tensor_copy(rbs[0:DH, qh, :], rb[0:DH, :])
                        nc.vector.tensor_mul(
                            dst[0:DH, dc + qh * QH:dc + (qh + 1) * QH],
                            ot[0:DH, qh, :], rbs[0:DH, qh, :],
                        )
                        if qh == 0 and between is not None:
                            between()
                    if h == 1:
                        nc.sync.dma_start(
                            otn[DH:128, qt * 1024:(qt + 1) * 1024],
                            dst[0:DH, :],
                        )
                return emit

            def attn_pass(b, h, qt, otn, filler, norm_prev=None, takes=None):
                hs = h * DH
                qs = b * S + qt * 1024

                def sc_mm(kc):
                    ks = b * S + kc * 128
                    sc = pss.tile([128, 2, QH], f32, tag="sc", name="sc")
                    for qh in range(2):
                        nc.tensor.matmul(
                            sc[:, qh, :],
                            lhsT=kT_sb[hs:hs + DH, ks:ks + 128],
                            rhs=qT_sb[hs:hs + DH,
                                      qs + qh * QH:qs + (qh + 1) * QH],
                        )
                    return sc

                ot = pso.tile([128, 2, QH], f32, tag="ot", name="ot")
                sc = sc_mm(0)
                for kc in range(PK):
                    attn = atp.tile([128, 2, QH], bf16, tag="attn",
                                    name="attn")
                    nc.scalar.activation(attn, sc, Exp, scale=SCALE,
                                         bias=mb_sb[:, b, kc:kc + 1])
                    if kc < PK - 1:
                        sc = sc_mm(kc + 1)
                    if kc == 0 and norm_prev is not None:
                        norm_prev()
                    # O^T[d, q] += V[k, d]^T attn[k, q]; row 64 (the ones
                    # column) accumulates Z = sum_k attn[k, q].
                    for qh in range(2):
                        nc.tensor.matmul(
                            ot[0:VW, qh, :],
                            lhsT=v2_sb[:, b, kc, h, :],
                            rhs=attn[:, qh, :],
                            start=(kc == 0), stop=(kc == PK - 1),
                            skip_group_check=True,
                        )
                    filler.take(takes.get(kc, 1) if takes else 1)
                    if kc == 0 and norm_prev is not None:
                        filler.take(4)
                return make_norm(h, qt, ot, otn)

            # ---- batch 0 prologue -------------------------------------
            # Serial prefix = only what the first attention chunks need:
            # K slice 0, V chunks 0-3, Q slices 0-1. Everything else is
            # filler inside pass 1, paced to the staged DMA arrivals.
            nc.sync.dma_start(wk_sb, wk)
            xhk = xhp.tile([128, PD, S], bf16, tag="xh", name="xhk")
            xhv = xhp.tile([128, PD, S], bf16, tag="xh", name="xhv")
            xhq = xhp.tile([128, PD, S], bf16, tag="xh", name="xhq")
            emit_load("k", 0, xhk, 0, 512)
            nc.sync.dma_start(wq_sb, wq)
            nc.sync.dma_start(wv_sb, wv)
            nc.sync.dma_start(wo_sb, wo)
            nc.sync.dma_start(mb_sb, mb)
            # Tiny warm-up ops: let DVE/ACT observe the mask DMA early and
            # pull the ~2.7us exp table load off the critical path.
            scratch = wpool.tile([1, 2], f32)
            nc.vector.tensor_copy(scratch, mb_sb[0:1, 0, 0:2])
            scratch2 = wpool.tile([1, 2], f32)
            nc.scalar.activation(scratch2, mb_sb[0:1, 0, 0:2], Exp)
            emit_load("v", 0, xhv, 0, 512)
            emit_load("q", 0, xhq, 0, 512)
            emit_load("q", 0, xhq, 512, 512)
            emit_load("k", 0, xhk, 512, 512)
            emit_load("v", 0, xhv, 512, 512)
            emit_load("k", 0, xhk, 1024, 512)
            emit_load("v", 0, xhv, 1024, 512)
            emit_load("k", 0, xhk, 1536, 512)
            emit_load("v", 0, xhv, 1536, 512)
            emit_load("q", 0, xhq, 1024, 512)
            emit_load("q", 0, xhq, 1536, 512)

            run_gen(emit_group("k", 0, xhk, 0))
            run_gen(emit_group("v", 0, xhv, 0))
            run_gen(emit_group("q", 0, xhq, 0))
            run_gen(emit_group("q", 0, xhq, 1))

            def gen_b0_rest():
                yield from emit_group("k", 0, xhk, 1)
                yield from emit_group("v", 0, xhv, 1)
                yield from emit_group("k", 0, xhk, 2)
                yield from emit_group("v", 0, xhv, 2)
                yield from emit_group("k", 0, xhk, 3)
                yield from emit_group("v", 0, xhv, 3)
                yield from emit_group("q", 0, xhq, 2)
                yield from emit_group("q", 0, xhq, 3)

            # ---- attention, software-pipelined ------------------------
            fil = _Filler(gen_b0_rest(), gen_proj(1))
            otn0 = op.tile([128, S], bf16, tag="otn", name="otn0")
            otn1 = op.tile([128, S], bf16, tag="otn", name="otn1")
            # pass 1: K/V/Q tails land mid-pass; pace the filler to the
            # staged DMA arrivals.
            nrm = attn_pass(0, 0, 0, otn0, fil,
                            takes={0: 8, 1: 8, 2: 0, 3: 0, 4: 8, 5: 0,
                                   6: 8, 7: 0, 8: 0, 9: 8, 10: 0, 11: 8,
                                   12: 0, 13: 8, 14: 0, 15: 8})
            nrm = attn_pass(0, 0, 1, otn0, fil, norm_prev=nrm,
                            takes={kc: 2 if kc % 2 == 0 else 1
                                   for kc in range(PK)})
            nrm = attn_pass(0, 1, 0, otn0, fil, norm_prev=nrm,
                            takes={kc: 2 if kc % 2 == 0 else 1
                                   for kc in range(PK)})
            nrm = attn_pass(0, 1, 1, otn0, fil, norm_prev=nrm,
                            takes={kc: 2 for kc in range(PK)})

            # Batch 1 runs head 1 first so the LAST pass is head 0, whose
            # normalization writes otn directly -- no partition-shift DMA
            # on the tail critical path.
            fil.add(gen_outproj(0, otn0, 0, S // 128))
            nrm = attn_pass(1, 1, 0, otn1, fil, norm_prev=nrm,
                            takes={kc: 2 if kc < 8 else 1
                                   for kc in range(PK)})
            nrm = attn_pass(1, 1, 1, otn1, fil, norm_prev=nrm)
            nrm = attn_pass(1, 0, 0, otn1, fil, norm_prev=nrm)
            fil.add(gen_outproj(1, otn1, 0, 8))
            nrm = attn_pass(1, 0, 1, otn1, fil, norm_prev=nrm)

            # ---- tail: final normalization + remaining output proj ----
            # The first half of the remaining output projection depends
            # only on the qh=0 normalization half; interleave it so it
            # overlaps the qh=1 chain.
            nrm(between=lambda: run_gen(gen_outproj(1, otn1, 8, 12,
                                                    tail=True)))
            fil.add(gen_outproj(1, otn1, 12, S // 128, tail=True))
            fil.take(1 << 20)

    nc.compile()
    _NC_CACHE["nc"] = nc
    return nc


def _prep_inputs(queries, keys, values, masks, Wq, Wk, Wv, Wo):
    """Host-side sharding/layout prep. Returns per-core input maps."""
    def t_bf16(x):  # [B, S, D] -> [D, B*S] bf16, contiguous
        return np.ascontiguousarray(
            np.asarray(x, dtype=np.float32).reshape(BS, D).astype(BF16).T
        )

    xqT, xkT, xvT = t_bf16(queries), t_bf16(keys), t_bf16(values)

    m01 = (np.asarray(masks) != 0).astype(np.float32)          # [B, S]
    mb = np.ascontiguousarray(
        np.where(m01.reshape(B, PK, 128) != 0, 0.0, -30000.0)
        .transpose(2, 0, 1).astype(np.float32)
    )

    def w_prep(W, c):  # [D, D] -> [128, PD, MH] bf16 slice for core c
        Wc = np.asarray(W, dtype=np.float32)[:, c * MH:(c + 1) * MH]
        return np.ascontiguousarray(
            Wc.astype(BF16).reshape(PD, 128, MH).transpose(1, 0, 2)
        )

    Wo_f = np.asarray(Wo, dtype=np.float32)
    in_maps = []
    for c in range(NCORES):
        in_maps.append({
            "xqT": xqT, "xkT": xkT, "xvT": xvT,
            "wq": w_prep(Wq, c), "wk": w_prep(Wk, c), "wv": w_prep(Wv, c),
            "wo": np.ascontiguousarray(
                Wo_f[c * MH:(c + 1) * MH, :].astype(BF16)
            ),
            "mb": mb,
        })
    return in_maps


def run(inputs, trace=False, trace_cores=None):
    """Run on 8 NeuronCores; returns (output [B,S,D] f32, BassKernelResults)."""
    from concourse.bass_utils import run_bass_kernel_spmd

    nc = _build_nc()
    in_maps = _prep_inputs(**inputs)
    res = run_bass_kernel_spmd(
        nc, in_maps, core_ids=list(range(NCORES)),
        trace=trace, trace_cores=trace_cores,
    )
    acc = res.results[0]["out"].astype(np.float32, copy=True)
    for r in res.results[1:]:
        acc += r["out"]
    return acc.reshape(B, S, D), res


def kernel(**inputs) -> np.ndarray:
    out, _ = run(inputs)
    return out


# revision 65
# speedup vs baseline: 1.0023x; 1.0005x over previous
"""Multi-head attention (B=2, S=2048, D=1024, H=16) on 8 Trainium2 cores.

Sharding: head-parallel. Core c owns heads {2c, 2c+1} (a contiguous
128-wide slice of the projection space). Each core reads the full
(transposed, bf16) activations, computes its heads' Q/K/V projections,
full S x S attention, and its partial contribution to the output
projection (row-parallel Wo). Host sums the 8 partials.

Device-side layout notes:
  - Scores are computed transposed (scoresT[k, q]) so the softmax
    contraction (over k) lands on the PSUM partition axis, where the
    tensor engine can both re-sum it and contract it with V.
  - Attention runs one head at a time over 1024-wide q tiles, one
    matmul per 128-key chunk (N=1024 spanning two PSUM banks), so each
    chunk parks at most one attn@V matmul in the PE wait queue and the
    exp stream on the scalar engine never stalls on sequencer backup.
  - V carries an extra all-ones 65th column, so the attn@V matmul
    (M=65) also accumulates the softmax denominator Z into PSUM row 64
    for free -- no separate denominator matmuls on the PE.
  - The key-padding mask is a per-partition (per-key) bias of -30000
    applied inside the exp activation (out = exp(in*scale + bias)), so
    masked keys' attention weights are exactly 0 at zero extra cost.
  - V is projected directly in [key-row, dim] orientation (no DMA
    transposes), copied into a per-head [V | ones] SBUF layout.
  - 1/Z (one partition row) is broadcast across the head's 64
    partitions by K=1 bf16 ones-matmuls, staged through SBUF (DVE
    reads at most one PSUM operand); tensor_muls write the normalized
    O^T half into the stacked [128, S] otn used as lhsT by the K=128
    output projection. Head 1's half reaches partitions 64:128 via a
    partition-shifting DMA copy (engines cannot shift partitions).
  - Software pipelining against the in-order PE queue: the tail of
    batch 0's projections, all of batch 1's projections, and both
    output projections are emitted as filler inside the (exp-paced)
    attention chunk loops, so the PE stays busy while the scalar
    engine works through the exps. Pass-i's normalization is emitted
    inside pass i+1 so the reciprocal latency never stalls the exps.
  - Output partials are stored fp16; the host accumulates in fp32.
    Stores ride the GPSIMD (Pool) SWDGE queue so they never
    head-of-line block the activation loads on the SP queue.
"""

import math

import ml_dtypes
import numpy as np

B, S, D, H = 2, 2048, 1024, 16
DH = D // H            # 64
NCORES = 8
MH = 2 * DH            # 128: per-core slice of the head dim (2 heads)
BS = B * S             # 4096
PK = S // 128          # 16 key chunks per batch
PD = D // 128          # 8 contraction chunks for the projections
QT = 512               # projection q-tile width
QH = 512               # attention q half-tile (PSUM bank width)
VW = DH + 1            # 65: V columns incl. the ones column
SCALE = 1.0 / math.sqrt(DH)
BF16 = ml_dtypes.bfloat16

_NC_CACHE = {}


class _Filler:
    """FIFO of generators that emit PE-filler instructions on demand."""

    def __init__(self, *gens):
        self.gens = list(gens)

    def add(self, gen):
        self.gens.append(gen)

    def take(self, n):
        while n > 0 and self.gens:
            try:
                next(self.gens[0])
                n -= 1
            except StopIteration:
                self.gens.pop(0)


def _build_nc():
    """Build the (core-independent) Bass program once."""
    if "nc" in _NC_CACHE:
        return _NC_CACHE["nc"]

    from contextlib import ExitStack

    import concourse.bacc as bacc
    import concourse.mybir as mybir
    import concourse.tile as tile

    f32 = mybir.dt.float32
    f32r = mybir.dt.float32r
    f16 = mybir.dt.float16
    bf16 = mybir.dt.bfloat16
    Exp = mybir.ActivationFunctionType.Exp

    nc = bacc.Bacc("TRN2", target_bir_lowering=False, debug=False)

    xqT = nc.dram_tensor("xqT", [D, BS], bf16, kind="ExternalInput").ap()
    xkT = nc.dram_tensor("xkT", [D, BS], bf16, kind="ExternalInput").ap()
    xvT = nc.dram_tensor("xvT", [D, BS], bf16, kind="ExternalInput").ap()
    wq = nc.dram_tensor("wq", [128, PD, MH], bf16, kind="ExternalInput").ap()
    wk = nc.dram_tensor("wk", [128, PD, MH], bf16, kind="ExternalInput").ap()
    wv = nc.dram_tensor("wv", [128, PD, MH], bf16, kind="ExternalInput").ap()
    wo = nc.dram_tensor("wo", [128, D], bf16, kind="ExternalInput").ap()
    mb = nc.dram_tensor("mb", [128, B, PK], f32, kind="ExternalInput").ap()
    out = nc.dram_tensor("out", [BS, D], f16, kind="ExternalOutput").ap()

    with tile.TileContext(nc) as tc, ExitStack() as ctx:
        wpool = ctx.enter_context(tc.tile_pool(name="wpool", bufs=1))
        apool = ctx.enter_context(tc.tile_pool(name="apool", bufs=1))

        wq_sb = wpool.tile([128, PD, MH], bf16)
        wk_sb = wpool.tile([128, PD, MH], bf16)
        wv_sb = wpool.tile([128, PD, MH], bf16)
        wo_sb = wpool.tile([128, D], bf16)
        mb_sb = wpool.tile([128, B, PK], f32)
        onesb_sb = wpool.tile([128, DH], bf16)
        nc.vector.memset(onesb_sb, 1.0)

        # Persistent per-core activations:
        #   qT_sb/kT_sb/vT_sb: [128 = 2 heads x 64 head-dims, BS] transposed
        #   v2_sb: [128 key positions, b, chunk, head, 64 dims + ones col]
        qT_sb = apool.tile([128, BS], bf16)
        kT_sb = apool.tile([128, BS], bf16)
        v2_sb = apool.tile([128, B, PK, 2, VW], bf16)
        nc.vector.memset(v2_sb[:, :, :, :, DH:DH + 1], 1.0)

        xhp = ctx.enter_context(tc.tile_pool(name="xhp", bufs=3))
        with (
            tc.tile_pool(name="atp", bufs=8) as atp,
            tc.tile_pool(name="rzp", bufs=2) as rzp,
            tc.tile_pool(name="op", bufs=2) as op,
            tc.tile_pool(name="outp", bufs=4) as outp,
            tc.tile_pool(name="psp", bufs=2, space="PSUM") as psp,
            tc.tile_pool(name="pss", bufs=2, space="PSUM") as pss,
            tc.tile_pool(name="pso", bufs=1, space="PSUM") as pso,
        ):
            TENSORS = {"q": (xqT, wq_sb, qT_sb), "k": (xkT, wk_sb, kT_sb),
                       "v": (xvT, wv_sb, None)}

            def emit_load(t, b, xh, cs0, cw, split=False):
                xT = TENSORS[t][0]
                src = xT[:, b * S + cs0:b * S + cs0 + cw]
                src = src.rearrange("(kc p) c -> p kc c", p=128)
                if split:
                    # Startup-critical loads: deliver the contraction
                    # chunks in two halves so the (accumulating) group's
                    # first matmuls start as soon as half the bytes land.
                    h = PD // 2
                    nc.sync.dma_start(xh[:, 0:h, cs0:cs0 + cw],
                                      src[:, 0:h, :])
                    nc.sync.dma_start(xh[:, h:PD, cs0:cs0 + cw],
                                      src[:, h:PD, :])
                else:
                    nc.sync.dma_start(xh[:, :, cs0:cs0 + cw], src)

            def emit_group(t, b, xh, sti):
                if t == "v":
                    # V is projected directly in [key-row, dim] orientation
                    # (no transposes): per 128-key chunk, 8 accumulating
                    # N=128 matmuls, then a strided copy into v2.
                    for kci in range(sti * 4, sti * 4 + 4):
                        ks = kci * 128
                        pv = psp.tile([128, QT], f32, tag="pq", name="pv")
                        for cc in range(PD):
                            nc.tensor.matmul(
                                pv[:, 0:128],
                                lhsT=xh[:, cc, ks:ks + 128],
                                rhs=wv_sb[:, cc, :],
                                start=(cc == 0),
                                stop=(cc == PD - 1),
                            )
                            if cc % 4 == 3:
                                yield
                        nc.vector.tensor_copy(
                            v2_sb[:, b, kci, :, 0:DH], pv[:, 0:128]
                        )
                    return
                w_sb, dst = TENSORS[t][1], TENSORS[t][2]
                pq = psp.tile([128, QT], f32, tag="pq", name="pq")
                for kc in range(PD):
                    nc.tensor.matmul(
                        pq,
                        lhsT=w_sb[:, kc, :],
                        rhs=xh[:, kc, sti * QT:(sti + 1) * QT],
                        start=(kc == 0),
                        stop=(kc == PD - 1),
                    )
                    yield
                ds = b * S + sti * QT
                nc.vector.tensor_copy(dst[:, ds:ds + QT], pq)

            def run_gen(g):
                for _ in g:
                    pass

            def gen_proj(b):
                for t in ("q", "k", "v"):
                    xh = xhp.tile([128, PD, S], bf16, tag="xh", name="xh")
                    emit_load(t, b, xh, 0, 1024)
                    emit_load(t, b, xh, 1024, 1024)
                    for sti in range(S // QT):
                        yield from emit_group(t, b, xh, sti)

            def gen_outproj(b, otn, st_lo, st_hi, tail=False):
                # In the tail (no exps left) the score PSUM ring is free:
                # use its wider slots and split the copies across DVE and
                # ACT so the psum-ring turnaround never paces the drain.
                for st in range(st_lo, st_hi):
                    rs = b * S + st * 128
                    ws = outp.tile([128, D], f16, tag="ws", name="ws")
                    if tail:
                        if st % 2 == 0:
                            wp2 = pss.tile([128, 2, QH], f32, tag="sc",
                                           name="wp2")
                            for nt in range(2):
                                nc.tensor.matmul(
                                    wp2[:, nt, :],
                                    lhsT=otn[:, st * 128:(st + 1) * 128],
                                    rhs=wo_sb[:, nt * QH:(nt + 1) * QH],
                                )
                                yield
                            nc.vector.tensor_copy(ws[:, 0:QH], wp2[:, 0, :])
                            nc.scalar.copy(ws[:, QH:D], wp2[:, 1, :])
                            # Alternate store queues so the serial SWDGE
                            # generation never paces the drain.
                            nc.sync.dma_start(out[rs:rs + 128, :], ws)
                            continue
                        for nt in range(2):
                            wp = psp.tile([128, QH], f32, tag="pq",
                                          name="wp")
                            nc.tensor.matmul(
                                wp,
                                lhsT=otn[:, st * 128:(st + 1) * 128],
                                rhs=wo_sb[:, nt * QH:(nt + 1) * QH],
                            )
                            yield
                            if nt == 0:
                                nc.vector.tensor_copy(ws[:, 0:QH], wp)
                            else:
                                nc.scalar.copy(ws[:, QH:D], wp)
                    else:
                        for nt in range(2):
                            wp = psp.tile([128, QH], f32, tag="pq",
                                          name="wp")
                            nc.tensor.matmul(
                                wp,
                                lhsT=otn[:, st * 128:(st + 1) * 128],
                                rhs=wo_sb[:, nt * QH:(nt + 1) * QH],
                            )
                            yield
                            nc.vector.tensor_copy(
                                ws[:, nt * QH:(nt + 1) * QH], wp
                            )
                    nc.gpsimd.dma_start(out[rs:rs + 128, :], ws)

            def make_norm(h, qt, ot, otn):
                # 1/Z on the single Z partition row, broadcast across the
                # head's 64 partitions via K=1 fp32r matmuls, staged
                # through SBUF (DVE ops keep one PSUM operand max), then
                # the normalized O^T half lands in otn -- directly for
                # head 0; via a partition-shifting DMA copy for head 1
                # (engines cannot shift partitions; the DMA can).
                # Emitted deferred -- inside the NEXT pass, after its
                # first two score matmuls -- so the reciprocal latency
                # never delays the exp stream.
                def emit(between=None):
                    rz = rzp.tile([128, 2, QH], bf16, tag="rz", name="rz")
                    rbs = rzp.tile([128, 2, QH], f32, tag="rbs", name="rbs")
                    dst = otn if h == 0 else op.tile(
                        [128, 1024], bf16, tag="ott", name="ott", bufs=2
                    )
                    dc = qt * 1024 if h == 0 else 0
                    with nc.allow_low_precision("1/Z broadcast in bf16"):
                        nc.vector.reciprocal(
                            rz[DH:DH + 1, :, :], ot[DH:DH + 1, :, :]
                        )
                    for qh in range(2):
                        rb = psp.tile([128, QH], f32, tag="pq", name="rb")
                        nc.tensor.matmul(
                            rb[0:DH, :],
                            lhsT=onesb_sb[DH:DH + 1, 0:DH],
                            rhs=rz[DH:DH + 1, qh, :],
                        )
                        # Stage 1/Z through SBUF: a DVE tensor op may read
                        # at most one PSUM operand.
                        nc.vector.tensor_copy(rbs[0:DH, qh, :], rb[0:DH, :])
                        nc.vector.tensor_mul(
                            dst[0:DH, dc + qh * QH:dc + (qh + 1) * QH],
                            ot[0:DH, qh, :], rbs[0:DH, qh, :],
                        )
                        if qh == 0 and between is not None:
                            between()
                    if h == 1:
                        nc.sync.dma_start(
                            otn[DH:128, qt * 1024:(qt + 1) * 1024],
                            dst[0:DH, :],
                        )
                return emit

            def attn_pass(b, h, qt, otn, filler, norm_prev=None, takes=None):
                hs = h * DH
                qs = b * S + qt * 1024

                def sc_mm(kc):
                    ks = b * S + kc * 128
                    sc = pss.tile([128, 2, QH], f32, tag="sc", name="sc")
                    for qh in range(2):
                        nc.tensor.matmul(
                            sc[:, qh, :],
                            lhsT=kT_sb[hs:hs + DH, ks:ks + 128],
                            rhs=qT_sb[hs:hs + DH,
                                      qs + qh * QH:qs + (qh + 1) * QH],
                        )
                    return sc

                ot = pso.tile([128, 2, QH], f32, tag="ot", name="ot")
                sc = sc_mm(0)
                for kc in range(PK):
                    attn = atp.tile([128, 2, QH], bf16, tag="attn",
                                    name="attn")
                    nc.scalar.activation(attn, sc, Exp, scale=SCALE,
                                         bias=mb_sb[:, b, kc:kc + 1])
                    if kc < PK - 1:
                        sc = sc_mm(kc + 1)
                    if kc == 0 and norm_prev is not None:
                        norm_prev()
                    # O^T[d, q] += V[k, d]^T attn[k, q]; row 64 (the ones
                    # column) accumulates Z = sum_k attn[k, q].
                    for qh in range(2):
                        nc.tensor.matmul(
                            ot[0:VW, qh, :],
                            lhsT=v2_sb[:, b, kc, h, :],
                            rhs=attn[:, qh, :],
                            start=(kc == 0), stop=(kc == PK - 1),
                            skip_group_check=True,
                        )
                    filler.take(takes.get(kc, 1) if takes else 1)
                    if kc == 0 and norm_prev is not None:
                        filler.take(4)
                return make_norm(h, qt, ot, otn)

            # ---- batch 0 prologue -------------------------------------
            # Serial prefix = only what the first attention chunks need:
            # K slice 0, V chunks 0-3, Q slices 0-1. Everything else is
            # filler inside pass 1, paced to the staged DMA arrivals.
            nc.sync.dma_start(wk_sb, wk)
            xhk = xhp.tile([128, PD, S], bf16, tag="xh", name="xhk")
            xhv = xhp.tile([128, PD, S], bf16, tag="xh", name="xhv")
            xhq = xhp.tile([128, PD, S], bf16, tag="xh", name="xhq")
            emit_load("k", 0, xhk, 0, 512, split=True)
            nc.sync.dma_start(wq_sb, wq)
            nc.sync.dma_start(wv_sb, wv)
            nc.sync.dma_start(wo_sb, wo)
            nc.sync.dma_start(mb_sb, mb)
            # Tiny warm-up ops: let DVE/ACT observe the mask DMA early and
            # pull the ~2.7us exp table load off the critical path.
            scratch = wpool.tile([1, 2], f32)
            nc.vector.tensor_copy(scratch, mb_sb[0:1, 0, 0:2])
            scratch2 = wpool.tile([1, 2], f32)
            nc.scalar.activation(scratch2, mb_sb[0:1, 0, 0:2], Exp)
            emit_load("v", 0, xhv, 0, 512, split=True)
            emit_load("q", 0, xhq, 0, 512, split=True)
            emit_load("q", 0, xhq, 512, 512, split=True)
            emit_load("k", 0, xhk, 512, 512)
            emit_load("v", 0, xhv, 512, 512)
            emit_load("k", 0, xhk, 1024, 512)
            emit_load("v", 0, xhv, 1024, 512)
            emit_load("k", 0, xhk, 1536, 512)
            emit_load("v", 0, xhv, 1536, 512)
            emit_load("q", 0, xhq, 1024, 512)
            emit_load("q", 0, xhq, 1536, 512)

            run_gen(emit_group("k", 0, xhk, 0))
            run_gen(emit_group("v", 0, xhv, 0))
            run_gen(emit_group("q", 0, xhq, 0))
            run_gen(emit_group("q", 0, xhq, 1))

            def gen_b0_rest():
                yield from emit_group("k", 0, xhk, 1)
                yield from emit_group("v", 0, xhv, 1)
                yield from emit_group("k", 0, xhk, 2)
                yield from emit_group("v", 0, xhv, 2)
                yield from emit_group("k", 0, xhk, 3)
                yield from emit_group("v", 0, xhv, 3)
                yield from emit_group("q", 0, xhq, 2)
                yield from emit_group("q", 0, xhq, 3)

            # ---- attention, software-pipelined ------------------------
            fil = _Filler(gen_b0_rest(), gen_proj(1))
            otn0 = op.tile([128, S], bf16, tag="otn", name="otn0")
            otn1 = op.tile([128, S], bf16, tag="otn", name="otn1")
            # pass 1: K/V/Q tails land mid-pass; pace the filler to the
            # staged DMA arrivals.
            nrm = attn_pass(0, 0, 0, otn0, fil,
                            takes={0: 8, 1: 8, 2: 0, 3: 0, 4: 8, 5: 0,
                                   6: 8, 7: 0, 8: 0, 9: 8, 10: 0, 11: 8,
                                   12: 0, 13: 8, 14: 0, 15: 8})
            nrm = attn_pass(0, 0, 1, otn0, fil, norm_prev=nrm,
                            takes={kc: 2 if kc % 2 == 0 else 1
                                   for kc in range(PK)})
            nrm = attn_pass(0, 1, 0, otn0, fil, norm_prev=nrm,
                            takes={kc: 2 if kc % 2 == 0 else 1
                                   for kc in range(PK)})
            nrm = attn_pass(0, 1, 1, otn0, fil, norm_prev=nrm,
                            takes={kc: 2 for kc in range(PK)})

            # Batch 1 runs head 1 first so the LAST pass is head 0, whose
            # normalization writes otn directly -- no partition-shift DMA
            # on the tail critical path.
            fil.add(gen_outproj(0, otn0, 0, S // 128))
            nrm = attn_pass(1, 1, 0, otn1, fil, norm_prev=nrm,
                            takes={kc: 2 if kc < 8 else 1
                                   for kc in range(PK)})
            nrm = attn_pass(1, 1, 1, otn1, fil, norm_prev=nrm)
            nrm = attn_pass(1, 0, 0, otn1, fil, norm_prev=nrm)
            fil.add(gen_outproj(1, otn1, 0, 8))
            nrm = attn_pass(1, 0, 1, otn1, fil, norm_prev=nrm)

            # ---- tail: final normalization + remaining output proj ----
            # The first half of the remaining output projection depends
            # only on the qh=0 normalization half; interleave it so it
            # overlaps the qh=1 chain.
            nrm(between=lambda: run_gen(gen_outproj(1, otn1, 8, 12,
                                                    tail=True)))
            fil.add(gen_outproj(1, otn1, 12, S // 128, tail=True))
            fil.take(1 << 20)

    nc.compile()
    _NC_CACHE["nc"] = nc
    return nc


def _prep_inputs(queries, keys, values, masks, Wq, Wk, Wv, Wo):
    """Host-side sharding/layout prep. Returns per-core input maps."""
    def t_bf16(x):  # [B, S, D] -> [D, B*S] bf16, contiguous
        return np.ascontiguousarray(
            np.asarray(x, dtype=np.float32).reshape(BS, D).astype(BF16).T
        )

    xqT, xkT, xvT = t_bf16(queries), t_bf16(keys), t_bf16(values)

    m01 = (np.asarray(masks) != 0).astype(np.float32)          # [B, S]
    mb = np.ascontiguousarray(
        np.where(m01.reshape(B, PK, 128) != 0, 0.0, -30000.0)
        .transpose(2, 0, 1).astype(np.float32)
    )

    def w_prep(W, c):  # [D, D] -> [128, PD, MH] bf16 slice for core c
        Wc = np.asarray(W, dtype=np.float32)[:, c * MH:(c + 1) * MH]
        return np.ascontiguousarray(
            Wc.astype(BF16).reshape(PD, 128, MH).transpose(1, 0, 2)
        )

    Wo_f = np.asarray(Wo, dtype=np.float32)
    in_maps = []
    for c in range(NCORES):
        in_maps.append({
            "xqT": xqT, "xkT": xkT, "xvT": xvT,
            "wq": w_prep(Wq, c), "wk": w_prep(Wk, c), "wv": w_prep(Wv, c),
            "wo": np.ascontiguousarray(
                Wo_f[c * MH:(c + 1) * MH, :].astype(BF16)
            ),
            "mb": mb,
        })
    return in_maps


def run(inputs, trace=False, trace_cores=None):
    """Run on 8 NeuronCores; returns (output [B,S,D] f32, BassKernelResults)."""
    from concourse.bass_utils import run_bass_kernel_spmd

    nc = _build_nc()
    in_maps = _prep_inputs(**inputs)
    res = run_bass_kernel_spmd(
        nc, in_maps, core_ids=list(range(NCORES)),
        trace=trace, trace_cores=trace_cores,
    )
    acc = res.results[0]["out"].astype(np.float32, copy=True)
    for r in res.results[1:]:
        acc += r["out"]
    return acc.reshape(B, S, D), res


def kernel(**inputs) -> np.ndarray:
    out, _ = run(inputs)
    return out


# revision 66
# speedup vs baseline: 1.0062x; 1.0038x over previous
"""Multi-head attention (B=2, S=2048, D=1024, H=16) on 8 Trainium2 cores.

Sharding: head-parallel. Core c owns heads {2c, 2c+1} (a contiguous
128-wide slice of the projection space). Each core reads the full
(transposed, bf16) activations, computes its heads' Q/K/V projections,
full S x S attention, and its partial contribution to the output
projection (row-parallel Wo). Host sums the 8 partials.

Device-side layout notes:
  - Scores are computed transposed (scoresT[k, q]) so the softmax
    contraction (over k) lands on the PSUM partition axis, where the
    tensor engine can both re-sum it and contract it with V.
  - Attention runs one head at a time over 1024-wide q tiles, one
    matmul per 128-key chunk (N=1024 spanning two PSUM banks), so each
    chunk parks at most one attn@V matmul in the PE wait queue and the
    exp stream on the scalar engine never stalls on sequencer backup.
  - V carries an extra all-ones 65th column, so the attn@V matmul
    (M=65) also accumulates the softmax denominator Z into PSUM row 64
    for free -- no separate denominator matmuls on the PE.
  - The key-padding mask is a per-partition (per-key) bias of -30000
    applied inside the exp activation (out = exp(in*scale + bias)), so
    masked keys' attention weights are exactly 0 at zero extra cost.
  - V is projected directly in [key-row, dim] orientation (no DMA
    transposes), copied into a per-head [V | ones] SBUF layout.
  - 1/Z (one partition row) is broadcast across the head's 64
    partitions by K=1 bf16 ones-matmuls, staged through SBUF (DVE
    reads at most one PSUM operand); tensor_muls write the normalized
    O^T half into the stacked [128, S] otn used as lhsT by the K=128
    output projection. Head 1's half reaches partitions 64:128 via a
    partition-shifting DMA copy (engines cannot shift partitions).
  - Software pipelining against the in-order PE queue: the tail of
    batch 0's projections, all of batch 1's projections, and both
    output projections are emitted as filler inside the (exp-paced)
    attention chunk loops, so the PE stays busy while the scalar
    engine works through the exps. Pass-i's normalization is emitted
    inside pass i+1 so the reciprocal latency never stalls the exps.
  - Output partials are stored fp16; the host accumulates in fp32.
    Stores ride the GPSIMD (Pool) SWDGE queue so they never
    head-of-line block the activation loads on the SP queue.
"""

import math

import ml_dtypes
import numpy as np

B, S, D, H = 2, 2048, 1024, 16
DH = D // H            # 64
NCORES = 8
MH = 2 * DH            # 128: per-core slice of the head dim (2 heads)
BS = B * S             # 4096
PK = S // 128          # 16 key chunks per batch
PD = D // 128          # 8 contraction chunks for the projections
QT = 512               # projection q-tile width
QH = 512               # attention q half-tile (PSUM bank width)
VW = DH + 1            # 65: V columns incl. the ones column
SCALE = 1.0 / math.sqrt(DH)
BF16 = ml_dtypes.bfloat16

_NC_CACHE = {}


class _Filler:
    """FIFO of generators that emit PE-filler instructions on demand."""

    def __init__(self, *gens):
        self.gens = list(gens)

    def add(self, gen):
        self.gens.append(gen)

    def take(self, n):
        while n > 0 and self.gens:
            try:
                next(self.gens[0])
                n -= 1
            except StopIteration:
                self.gens.pop(0)


def _build_nc():
    """Build the (core-independent) Bass program once."""
    if "nc" in _NC_CACHE:
        return _NC_CACHE["nc"]

    from contextlib import ExitStack

    import concourse.bacc as bacc
    import concourse.mybir as mybir
    import concourse.tile as tile

    f32 = mybir.dt.float32
    f32r = mybir.dt.float32r
    f16 = mybir.dt.float16
    bf16 = mybir.dt.bfloat16
    Exp = mybir.ActivationFunctionType.Exp

    nc = bacc.Bacc("TRN2", target_bir_lowering=False, debug=False)

    xqT = nc.dram_tensor("xqT", [D, BS], bf16, kind="ExternalInput").ap()
    xkT = nc.dram_tensor("xkT", [D, BS], bf16, kind="ExternalInput").ap()
    xvT = nc.dram_tensor("xvT", [D, BS], bf16, kind="ExternalInput").ap()
    wq = nc.dram_tensor("wq", [128, PD, MH], bf16, kind="ExternalInput").ap()
    wk = nc.dram_tensor("wk", [128, PD, MH], bf16, kind="ExternalInput").ap()
    wv = nc.dram_tensor("wv", [128, PD, MH], bf16, kind="ExternalInput").ap()
    wo = nc.dram_tensor("wo", [128, D], bf16, kind="ExternalInput").ap()
    mb = nc.dram_tensor("mb", [128, B, PK], f32, kind="ExternalInput").ap()
    out = nc.dram_tensor("out", [BS, D], f16, kind="ExternalOutput").ap()

    with tile.TileContext(nc) as tc, ExitStack() as ctx:
        wpool = ctx.enter_context(tc.tile_pool(name="wpool", bufs=1))
        apool = ctx.enter_context(tc.tile_pool(name="apool", bufs=1))

        wq_sb = wpool.tile([128, PD, MH], bf16)
        wk_sb = wpool.tile([128, PD, MH], bf16)
        wv_sb = wpool.tile([128, PD, MH], bf16)
        wo_sb = wpool.tile([128, D], bf16)
        mb_sb = wpool.tile([128, B, PK], f32)
        onesb_sb = wpool.tile([128, DH], bf16)
        nc.vector.memset(onesb_sb, 1.0)

        # Persistent per-core activations:
        #   qT_sb/kT_sb/vT_sb: [128 = 2 heads x 64 head-dims, BS] transposed
        #   v2_sb: [128 key positions, b, chunk, head, 64 dims + ones col]
        qT_sb = apool.tile([128, BS], bf16)
        kT_sb = apool.tile([128, BS], bf16)
        v2_sb = apool.tile([128, B, PK, 2, VW], bf16)
        nc.vector.memset(v2_sb[:, :, :, :, DH:DH + 1], 1.0)

        xhp = ctx.enter_context(tc.tile_pool(name="xhp", bufs=3))
        with (
            tc.tile_pool(name="atp", bufs=8) as atp,
            tc.tile_pool(name="rzp", bufs=2) as rzp,
            tc.tile_pool(name="op", bufs=2) as op,
            tc.tile_pool(name="outp", bufs=4) as outp,
            tc.tile_pool(name="psp", bufs=2, space="PSUM") as psp,
            tc.tile_pool(name="pss", bufs=2, space="PSUM") as pss,
            tc.tile_pool(name="pso", bufs=1, space="PSUM") as pso,
        ):
            TENSORS = {"q": (xqT, wq_sb, qT_sb), "k": (xkT, wk_sb, kT_sb),
                       "v": (xvT, wv_sb, None)}

            def emit_load(t, b, xh, cs0, cw, split=False):
                xT = TENSORS[t][0]
                src = xT[:, b * S + cs0:b * S + cs0 + cw]
                src = src.rearrange("(kc p) c -> p kc c", p=128)
                if split:
                    # Startup-critical loads: deliver the contraction
                    # chunks in two halves so the (accumulating) group's
                    # first matmuls start as soon as half the bytes land.
                    h = PD // 2
                    nc.sync.dma_start(xh[:, 0:h, cs0:cs0 + cw],
                                      src[:, 0:h, :])
                    nc.sync.dma_start(xh[:, h:PD, cs0:cs0 + cw],
                                      src[:, h:PD, :])
                else:
                    nc.sync.dma_start(xh[:, :, cs0:cs0 + cw], src)

            def emit_group(t, b, xh, sti):
                if t == "v":
                    # V is projected directly in [key-row, dim] orientation
                    # (no transposes): per 128-key chunk, 8 accumulating
                    # N=128 matmuls, then a strided copy into v2.
                    for kci in range(sti * 4, sti * 4 + 4):
                        ks = kci * 128
                        pv = psp.tile([128, QT], f32, tag="pq", name="pv")
                        for cc in range(PD):
                            nc.tensor.matmul(
                                pv[:, 0:128],
                                lhsT=xh[:, cc, ks:ks + 128],
                                rhs=wv_sb[:, cc, :],
                                start=(cc == 0),
                                stop=(cc == PD - 1),
                            )
                            if cc % 4 == 3:
                                yield
                        nc.vector.tensor_copy(
                            v2_sb[:, b, kci, :, 0:DH], pv[:, 0:128]
                        )
                    return
                w_sb, dst = TENSORS[t][1], TENSORS[t][2]
                pq = psp.tile([128, QT], f32, tag="pq", name="pq")
                for kc in range(PD):
                    nc.tensor.matmul(
                        pq,
                        lhsT=w_sb[:, kc, :],
                        rhs=xh[:, kc, sti * QT:(sti + 1) * QT],
                        start=(kc == 0),
                        stop=(kc == PD - 1),
                    )
                    yield
                ds = b * S + sti * QT
                nc.vector.tensor_copy(dst[:, ds:ds + QT], pq)

            def run_gen(g):
                for _ in g:
                    pass

            def gen_proj(b):
                for t in ("q", "k", "v"):
                    xh = xhp.tile([128, PD, S], bf16, tag="xh", name="xh")
                    emit_load(t, b, xh, 0, 1024, split=True)
                    emit_load(t, b, xh, 1024, 1024, split=True)
                    for sti in range(S // QT):
                        yield from emit_group(t, b, xh, sti)

            def gen_outproj(b, otn, st_lo, st_hi, tail=False):
                # In the tail (no exps left) the score PSUM ring is free:
                # use its wider slots and split the copies across DVE and
                # ACT so the psum-ring turnaround never paces the drain.
                for st in range(st_lo, st_hi):
                    rs = b * S + st * 128
                    ws = outp.tile([128, D], f16, tag="ws", name="ws")
                    if tail:
                        if st % 2 == 0:
                            wp2 = pss.tile([128, 2, QH], f32, tag="sc",
                                           name="wp2")
                            for nt in range(2):
                                nc.tensor.matmul(
                                    wp2[:, nt, :],
                                    lhsT=otn[:, st * 128:(st + 1) * 128],
                                    rhs=wo_sb[:, nt * QH:(nt + 1) * QH],
                                )
                                yield
                            nc.vector.tensor_copy(ws[:, 0:QH], wp2[:, 0, :])
                            nc.scalar.copy(ws[:, QH:D], wp2[:, 1, :])
                            # Alternate store queues so the serial SWDGE
                            # generation never paces the drain.
                            nc.sync.dma_start(out[rs:rs + 128, :], ws)
                            continue
                        for nt in range(2):
                            wp = psp.tile([128, QH], f32, tag="pq",
                                          name="wp")
                            nc.tensor.matmul(
                                wp,
                                lhsT=otn[:, st * 128:(st + 1) * 128],
                                rhs=wo_sb[:, nt * QH:(nt + 1) * QH],
                            )
                            yield
                            if nt == 0:
                                nc.vector.tensor_copy(ws[:, 0:QH], wp)
                            else:
                                nc.scalar.copy(ws[:, QH:D], wp)
                    else:
                        for nt in range(2):
                            wp = psp.tile([128, QH], f32, tag="pq",
                                          name="wp")
                            nc.tensor.matmul(
                                wp,
                                lhsT=otn[:, st * 128:(st + 1) * 128],
                                rhs=wo_sb[:, nt * QH:(nt + 1) * QH],
                            )
                            yield
                            nc.vector.tensor_copy(
                                ws[:, nt * QH:(nt + 1) * QH], wp
                            )
                    nc.gpsimd.dma_start(out[rs:rs + 128, :], ws)

            def make_norm(h, qt, ot, otn):
                # 1/Z on the single Z partition row, broadcast across the
                # head's 64 partitions via K=1 fp32r matmuls, staged
                # through SBUF (DVE ops keep one PSUM operand max), then
                # the normalized O^T half lands in otn -- directly for
                # head 0; via a partition-shifting DMA copy for head 1
                # (engines cannot shift partitions; the DMA can).
                # Emitted deferred -- inside the NEXT pass, after its
                # first two score matmuls -- so the reciprocal latency
                # never delays the exp stream.
                def emit(between=None):
                    rz = rzp.tile([128, 2, QH], bf16, tag="rz", name="rz")
                    rbs = rzp.tile([128, 2, QH], f32, tag="rbs", name="rbs")
                    dst = otn if h == 0 else op.tile(
                        [128, 1024], bf16, tag="ott", name="ott", bufs=2
                    )
                    dc = qt * 1024 if h == 0 else 0
                    with nc.allow_low_precision("1/Z broadcast in bf16"):
                        nc.vector.reciprocal(
                            rz[DH:DH + 1, :, :], ot[DH:DH + 1, :, :]
                        )
                    for qh in range(2):
                        rb = psp.tile([128, QH], f32, tag="pq", name="rb")
                        nc.tensor.matmul(
                            rb[0:DH, :],
                            lhsT=onesb_sb[DH:DH + 1, 0:DH],
                            rhs=rz[DH:DH + 1, qh, :],
                        )
                        # Stage 1/Z through SBUF: a DVE tensor op may read
                        # at most one PSUM operand.
                        nc.vector.tensor_copy(rbs[0:DH, qh, :], rb[0:DH, :])
                        nc.vector.tensor_mul(
                            dst[0:DH, dc + qh * QH:dc + (qh + 1) * QH],
                            ot[0:DH, qh, :], rbs[0:DH, qh, :],
                        )
                        if qh == 0 and between is not None:
                            between()
                    if h == 1:
                        nc.sync.dma_start(
                            otn[DH:128, qt * 1024:(qt + 1) * 1024],
                            dst[0:DH, :],
                        )
                return emit

            def attn_pass(b, h, qt, otn, filler, norm_prev=None, takes=None):
                hs = h * DH
                qs = b * S + qt * 1024

                def sc_mm(kc):
                    ks = b * S + kc * 128
                    sc = pss.tile([128, 2, QH], f32, tag="sc", name="sc")
                    for qh in range(2):
                        nc.tensor.matmul(
                            sc[:, qh, :],
                            lhsT=kT_sb[hs:hs + DH, ks:ks + 128],
                            rhs=qT_sb[hs:hs + DH,
                                      qs + qh * QH:qs + (qh + 1) * QH],
                        )
                    return sc

                ot = pso.tile([128, 2, QH], f32, tag="ot", name="ot")
                sc = sc_mm(0)
                for kc in range(PK):
                    attn = atp.tile([128, 2, QH], bf16, tag="attn",
                                    name="attn")
                    nc.scalar.activation(attn, sc, Exp, scale=SCALE,
                                         bias=mb_sb[:, b, kc:kc + 1])
                    if kc < PK - 1:
                        sc = sc_mm(kc + 1)
                    if kc == 0 and norm_prev is not None:
                        norm_prev()
                    # O^T[d, q] += V[k, d]^T attn[k, q]; row 64 (the ones
                    # column) accumulates Z = sum_k attn[k, q].
                    for qh in range(2):
                        nc.tensor.matmul(
                            ot[0:VW, qh, :],
                            lhsT=v2_sb[:, b, kc, h, :],
                            rhs=attn[:, qh, :],
                            start=(kc == 0), stop=(kc == PK - 1),
                            skip_group_check=True,
                        )
                    filler.take(takes.get(kc, 1) if takes else 1)
                    if kc == 0 and norm_prev is not None:
                        filler.take(4)
                return make_norm(h, qt, ot, otn)

            # ---- batch 0 prologue -------------------------------------
            # Serial prefix = only what the first attention chunks need:
            # K slice 0, V chunks 0-3, Q slices 0-1. Everything else is
            # filler inside pass 1, paced to the staged DMA arrivals.
            nc.sync.dma_start(wk_sb, wk)
            xhk = xhp.tile([128, PD, S], bf16, tag="xh", name="xhk")
            xhv = xhp.tile([128, PD, S], bf16, tag="xh", name="xhv")
            xhq = xhp.tile([128, PD, S], bf16, tag="xh", name="xhq")
            emit_load("k", 0, xhk, 0, 512, split=True)
            nc.sync.dma_start(wq_sb, wq)
            nc.sync.dma_start(wv_sb, wv)
            nc.sync.dma_start(wo_sb, wo)
            nc.sync.dma_start(mb_sb, mb)
            # Tiny warm-up ops: let DVE/ACT observe the mask DMA early and
            # pull the ~2.7us exp table load off the critical path.
            scratch = wpool.tile([1, 2], f32)
            nc.vector.tensor_copy(scratch, mb_sb[0:1, 0, 0:2])
            scratch2 = wpool.tile([1, 2], f32)
            nc.scalar.activation(scratch2, mb_sb[0:1, 0, 0:2], Exp)
            emit_load("v", 0, xhv, 0, 512, split=True)
            emit_load("q", 0, xhq, 0, 512, split=True)
            emit_load("q", 0, xhq, 512, 512, split=True)
            emit_load("k", 0, xhk, 512, 512, split=True)
            emit_load("v", 0, xhv, 512, 512, split=True)
            emit_load("k", 0, xhk, 1024, 512, split=True)
            emit_load("v", 0, xhv, 1024, 512, split=True)
            emit_load("k", 0, xhk, 1536, 512, split=True)
            emit_load("v", 0, xhv, 1536, 512, split=True)
            emit_load("q", 0, xhq, 1024, 512, split=True)
            emit_load("q", 0, xhq, 1536, 512, split=True)

            run_gen(emit_group("k", 0, xhk, 0))
            run_gen(emit_group("v", 0, xhv, 0))
            run_gen(emit_group("q", 0, xhq, 0))
            run_gen(emit_group("q", 0, xhq, 1))

            def gen_b0_rest():
                yield from emit_group("k", 0, xhk, 1)
                yield from emit_group("v", 0, xhv, 1)
                yield from emit_group("k", 0, xhk, 2)
                yield from emit_group("v", 0, xhv, 2)
                yield from emit_group("k", 0, xhk, 3)
                yield from emit_group("v", 0, xhv, 3)
                yield from emit_group("q", 0, xhq, 2)
                yield from emit_group("q", 0, xhq, 3)

            # ---- attention, software-pipelined ------------------------
            fil = _Filler(gen_b0_rest(), gen_proj(1))
            otn0 = op.tile([128, S], bf16, tag="otn", name="otn0")
            otn1 = op.tile([128, S], bf16, tag="otn", name="otn1")
            # pass 1: K/V/Q tails land mid-pass; pace the filler to the
            # staged DMA arrivals.
            nrm = attn_pass(0, 0, 0, otn0, fil,
                            takes={0: 8, 1: 8, 2: 0, 3: 0, 4: 8, 5: 0,
                                   6: 8, 7: 0, 8: 0, 9: 8, 10: 0, 11: 8,
                                   12: 0, 13: 8, 14: 0, 15: 8})
            nrm = attn_pass(0, 0, 1, otn0, fil, norm_prev=nrm,
                            takes={kc: 2 if kc % 2 == 0 else 1
                                   for kc in range(PK)})
            nrm = attn_pass(0, 1, 0, otn0, fil, norm_prev=nrm,
                            takes={kc: 2 if kc % 2 == 0 else 1
                                   for kc in range(PK)})
            nrm = attn_pass(0, 1, 1, otn0, fil, norm_prev=nrm,
                            takes={kc: 2 for kc in range(PK)})

            # Batch 1 runs head 1 first so the LAST pass is head 0, whose
            # normalization writes otn directly -- no partition-shift DMA
            # on the tail critical path.
            fil.add(gen_outproj(0, otn0, 0, S // 128))
            nrm = attn_pass(1, 1, 0, otn1, fil, norm_prev=nrm,
                            takes={kc: 2 if kc < 8 else 1
                                   for kc in range(PK)})
            nrm = attn_pass(1, 1, 1, otn1, fil, norm_prev=nrm)
            nrm = attn_pass(1, 0, 0, otn1, fil, norm_prev=nrm)
            fil.add(gen_outproj(1, otn1, 0, 8))
            nrm = attn_pass(1, 0, 1, otn1, fil, norm_prev=nrm)

            # ---- tail: final normalization + remaining output proj ----
            # The first half of the remaining output projection depends
            # only on the qh=0 normalization half; interleave it so it
            # overlaps the qh=1 chain.
            nrm(between=lambda: run_gen(gen_outproj(1, otn1, 8, 12,
                                                    tail=True)))
            fil.add(gen_outproj(1, otn1, 12, S // 128, tail=True))
            fil.take(1 << 20)

    nc.compile()
    _NC_CACHE["nc"] = nc
    return nc


def _prep_inputs(queries, keys, values, masks, Wq, Wk, Wv, Wo):
    """Host-side sharding/layout prep. Returns per-core input maps."""
    def t_bf16(x):  # [B, S, D] -> [D, B*S] bf16, contiguous
        return np.ascontiguousarray(
            np.asarray(x, dtype=np.float32).reshape(BS, D).astype(BF16).T
        )

    xqT, xkT, xvT = t_bf16(queries), t_bf16(keys), t_bf16(values)

    m01 = (np.asarray(masks) != 0).astype(np.float32)          # [B, S]
    mb = np.ascontiguousarray(
        np.where(m01.reshape(B, PK, 128) != 0, 0.0, -30000.0)
        .transpose(2, 0, 1).astype(np.float32)
    )

    def w_prep(W, c):  # [D, D] -> [128, PD, MH] bf16 slice for core c
        Wc = np.asarray(W, dtype=np.float32)[:, c * MH:(c + 1) * MH]
        return np.ascontiguousarray(
            Wc.astype(BF16).reshape(PD, 128, MH).transpose(1, 0, 2)
        )

    Wo_f = np.asarray(Wo, dtype=np.float32)
    in_maps = []
    for c in range(NCORES):
        in_maps.append({
            "xqT": xqT, "xkT": xkT, "xvT": xvT,
            "wq": w_prep(Wq, c), "wk": w_prep(Wk, c), "wv": w_prep(Wv, c),
            "wo": np.ascontiguousarray(
                Wo_f[c * MH:(c + 1) * MH, :].astype(BF16)
            ),
            "mb": mb,
        })
    return in_maps


def run(inputs, trace=False, trace_cores=None):
    """Run on 8 NeuronCores; returns (output [B,S,D] f32, BassKernelResults)."""
    from concourse.bass_utils import run_bass_kernel_spmd

    nc = _build_nc()
    in_maps = _prep_inputs(**inputs)
    res = run_bass_kernel_spmd(
        nc, in_maps, core_ids=list(range(NCORES)),
        trace=trace, trace_cores=trace_cores,
    )
    acc = res.results[0]["out"].astype(np.float32, copy=True)
    for r in res.results[1:]:
        acc += r["out"]
    return acc.reshape(B, S, D), res


def kernel(**inputs) -> np.ndarray:
    out, _ = run(inputs)
    return out


# revision 67
# speedup vs baseline: 1.0091x; 1.0029x over previous
"""Multi-head attention (B=2, S=2048, D=1024, H=16) on 8 Trainium2 cores.

Sharding: head-parallel. Core c owns heads {2c, 2c+1} (a contiguous
128-wide slice of the projection space). Each core reads the full
(transposed, bf16) activations, computes its heads' Q/K/V projections,
full S x S attention, and its partial contribution to the output
projection (row-parallel Wo). Host sums the 8 partials.

Device-side layout notes:
  - Scores are computed transposed (scoresT[k, q]) so the softmax
    contraction (over k) lands on the PSUM partition axis, where the
    tensor engine can both re-sum it and contract it with V.
  - Attention runs one head at a time over 1024-wide q tiles, one
    matmul per 128-key chunk (N=1024 spanning two PSUM banks), so each
    chunk parks at most one attn@V matmul in the PE wait queue and the
    exp stream on the scalar engine never stalls on sequencer backup.
  - V carries an extra all-ones 65th column, so the attn@V matmul
    (M=65) also accumulates the softmax denominator Z into PSUM row 64
    for free -- no separate denominator matmuls on the PE.
  - The key-padding mask is a per-partition (per-key) bias of -30000
    applied inside the exp activation (out = exp(in*scale + bias)), so
    masked keys' attention weights are exactly 0 at zero extra cost.
  - V is projected directly in [key-row, dim] orientation (no DMA
    transposes), copied into a per-head [V | ones] SBUF layout.
  - 1/Z (one partition row) is broadcast across the head's 64
    partitions by K=1 bf16 ones-matmuls, staged through SBUF (DVE
    reads at most one PSUM operand); tensor_muls write the normalized
    O^T half into the stacked [128, S] otn used as lhsT by the K=128
    output projection. Head 1's half reaches partitions 64:128 via a
    partition-shifting DMA copy (engines cannot shift partitions).
  - Software pipelining against the in-order PE queue: the tail of
    batch 0's projections, all of batch 1's projections, and both
    output projections are emitted as filler inside the (exp-paced)
    attention chunk loops, so the PE stays busy while the scalar
    engine works through the exps. Pass-i's normalization is emitted
    inside pass i+1 so the reciprocal latency never stalls the exps.
  - Output partials are stored fp16; the host accumulates in fp32.
    Stores ride the GPSIMD (Pool) SWDGE queue so they never
    head-of-line block the activation loads on the SP queue.
"""

import math

import ml_dtypes
import numpy as np

B, S, D, H = 2, 2048, 1024, 16
DH = D // H            # 64
NCORES = 8
MH = 2 * DH            # 128: per-core slice of the head dim (2 heads)
BS = B * S             # 4096
PK = S // 128          # 16 key chunks per batch
PD = D // 128          # 8 contraction chunks for the projections
QT = 512               # projection q-tile width
QH = 512               # attention q half-tile (PSUM bank width)
VW = DH + 1            # 65: V columns incl. the ones column
SCALE = 1.0 / math.sqrt(DH)
BF16 = ml_dtypes.bfloat16

_NC_CACHE = {}


class _Filler:
    """FIFO of generators that emit PE-filler instructions on demand."""

    def __init__(self, *gens):
        self.gens = list(gens)

    def add(self, gen):
        self.gens.append(gen)

    def take(self, n):
        while n > 0 and self.gens:
            try:
                next(self.gens[0])
                n -= 1
            except StopIteration:
                self.gens.pop(0)


def _build_nc():
    """Build the (core-independent) Bass program once."""
    if "nc" in _NC_CACHE:
        return _NC_CACHE["nc"]

    from contextlib import ExitStack

    import concourse.bacc as bacc
    import concourse.mybir as mybir
    import concourse.tile as tile

    f32 = mybir.dt.float32
    f32r = mybir.dt.float32r
    f16 = mybir.dt.float16
    bf16 = mybir.dt.bfloat16
    Exp = mybir.ActivationFunctionType.Exp

    nc = bacc.Bacc("TRN2", target_bir_lowering=False, debug=False)

    xqT = nc.dram_tensor("xqT", [D, BS], bf16, kind="ExternalInput").ap()
    xkT = nc.dram_tensor("xkT", [D, BS], bf16, kind="ExternalInput").ap()
    xvT = nc.dram_tensor("xvT", [D, BS], bf16, kind="ExternalInput").ap()
    wq = nc.dram_tensor("wq", [128, PD, MH], bf16, kind="ExternalInput").ap()
    wk = nc.dram_tensor("wk", [128, PD, MH], bf16, kind="ExternalInput").ap()
    wv = nc.dram_tensor("wv", [128, PD, MH], bf16, kind="ExternalInput").ap()
    wo = nc.dram_tensor("wo", [128, D], bf16, kind="ExternalInput").ap()
    mb = nc.dram_tensor("mb", [128, B, PK], f32, kind="ExternalInput").ap()
    out = nc.dram_tensor("out", [BS, D], f16, kind="ExternalOutput").ap()

    with tile.TileContext(nc) as tc, ExitStack() as ctx:
        wpool = ctx.enter_context(tc.tile_pool(name="wpool", bufs=1))
        apool = ctx.enter_context(tc.tile_pool(name="apool", bufs=1))

        wq_sb = wpool.tile([128, PD, MH], bf16)
        wk_sb = wpool.tile([128, PD, MH], bf16)
        wv_sb = wpool.tile([128, PD, MH], bf16)
        wo_sb = wpool.tile([128, D], bf16)
        mb_sb = wpool.tile([128, B, PK], f32)
        onesb_sb = wpool.tile([128, DH], bf16)
        nc.vector.memset(onesb_sb, 1.0)

        # Persistent per-core activations:
        #   qT_sb/kT_sb/vT_sb: [128 = 2 heads x 64 head-dims, BS] transposed
        #   v2_sb: [128 key positions, b, chunk, head, 64 dims + ones col]
        qT_sb = apool.tile([128, BS], bf16)
        kT_sb = apool.tile([128, BS], bf16)
        v2_sb = apool.tile([128, B, PK, 2, VW], bf16)
        nc.vector.memset(v2_sb[:, :, :, :, DH:DH + 1], 1.0)

        xhp = ctx.enter_context(tc.tile_pool(name="xhp", bufs=3))
        with (
            tc.tile_pool(name="atp", bufs=8) as atp,
            tc.tile_pool(name="rzp", bufs=2) as rzp,
            tc.tile_pool(name="op", bufs=2) as op,
            tc.tile_pool(name="outp", bufs=4) as outp,
            tc.tile_pool(name="psp", bufs=2, space="PSUM") as psp,
            tc.tile_pool(name="pss", bufs=2, space="PSUM") as pss,
            tc.tile_pool(name="pso", bufs=1, space="PSUM") as pso,
        ):
            TENSORS = {"q": (xqT, wq_sb, qT_sb), "k": (xkT, wk_sb, kT_sb),
                       "v": (xvT, wv_sb, None)}

            def emit_load(t, b, xh, cs0, cw, split=False):
                xT = TENSORS[t][0]
                src = xT[:, b * S + cs0:b * S + cs0 + cw]
                src = src.rearrange("(kc p) c -> p kc c", p=128)
                if split:
                    # Startup-critical loads: deliver the contraction
                    # chunks in quarters so the (accumulating) group's
                    # first matmuls start as soon as the first bytes land.
                    h = PD // 4
                    for qi in range(4):
                        nc.sync.dma_start(
                            xh[:, qi * h:(qi + 1) * h, cs0:cs0 + cw],
                            src[:, qi * h:(qi + 1) * h, :])
                else:
                    nc.sync.dma_start(xh[:, :, cs0:cs0 + cw], src)

            def emit_group(t, b, xh, sti):
                if t == "v":
                    # V is projected directly in [key-row, dim] orientation
                    # (no transposes): per 128-key chunk, 8 accumulating
                    # N=128 matmuls, then a strided copy into v2.
                    for kci in range(sti * 4, sti * 4 + 4):
                        ks = kci * 128
                        pv = psp.tile([128, QT], f32, tag="pq", name="pv")
                        for cc in range(PD):
                            nc.tensor.matmul(
                                pv[:, 0:128],
                                lhsT=xh[:, cc, ks:ks + 128],
                                rhs=wv_sb[:, cc, :],
                                start=(cc == 0),
                                stop=(cc == PD - 1),
                            )
                            if cc % 4 == 3:
                                yield
                        nc.vector.tensor_copy(
                            v2_sb[:, b, kci, :, 0:DH], pv[:, 0:128]
                        )
                    return
                w_sb, dst = TENSORS[t][1], TENSORS[t][2]
                pq = psp.tile([128, QT], f32, tag="pq", name="pq")
                for kc in range(PD):
                    nc.tensor.matmul(
                        pq,
                        lhsT=w_sb[:, kc, :],
                        rhs=xh[:, kc, sti * QT:(sti + 1) * QT],
                        start=(kc == 0),
                        stop=(kc == PD - 1),
                    )
                    yield
                ds = b * S + sti * QT
                nc.vector.tensor_copy(dst[:, ds:ds + QT], pq)

            def run_gen(g):
                for _ in g:
                    pass

            def gen_proj(b):
                for t in ("q", "k", "v"):
                    xh = xhp.tile([128, PD, S], bf16, tag="xh", name="xh")
                    emit_load(t, b, xh, 0, 1024, split=True)
                    emit_load(t, b, xh, 1024, 1024, split=True)
                    for sti in range(S // QT):
                        yield from emit_group(t, b, xh, sti)

            def gen_outproj(b, otn, st_lo, st_hi, tail=False):
                # In the tail (no exps left) the score PSUM ring is free:
                # use its wider slots and split the copies across DVE and
                # ACT so the psum-ring turnaround never paces the drain.
                for st in range(st_lo, st_hi):
                    rs = b * S + st * 128
                    ws = outp.tile([128, D], f16, tag="ws", name="ws")
                    if tail:
                        if st % 2 == 0:
                            wp2 = pss.tile([128, 2, QH], f32, tag="sc",
                                           name="wp2")
                            for nt in range(2):
                                nc.tensor.matmul(
                                    wp2[:, nt, :],
                                    lhsT=otn[:, st * 128:(st + 1) * 128],
                                    rhs=wo_sb[:, nt * QH:(nt + 1) * QH],
                                )
                                yield
                            nc.vector.tensor_copy(ws[:, 0:QH], wp2[:, 0, :])
                            nc.scalar.copy(ws[:, QH:D], wp2[:, 1, :])
                            # Alternate store queues so the serial SWDGE
                            # generation never paces the drain.
                            nc.sync.dma_start(out[rs:rs + 128, :], ws)
                            continue
                        for nt in range(2):
                            wp = psp.tile([128, QH], f32, tag="pq",
                                          name="wp")
                            nc.tensor.matmul(
                                wp,
                                lhsT=otn[:, st * 128:(st + 1) * 128],
                                rhs=wo_sb[:, nt * QH:(nt + 1) * QH],
                            )
                            yield
                            if nt == 0:
                                nc.vector.tensor_copy(ws[:, 0:QH], wp)
                            else:
                                nc.scalar.copy(ws[:, QH:D], wp)
                    else:
                        for nt in range(2):
                            wp = psp.tile([128, QH], f32, tag="pq",
                                          name="wp")
                            nc.tensor.matmul(
                                wp,
                                lhsT=otn[:, st * 128:(st + 1) * 128],
                                rhs=wo_sb[:, nt * QH:(nt + 1) * QH],
                            )
                            yield
                            nc.vector.tensor_copy(
                                ws[:, nt * QH:(nt + 1) * QH], wp
                            )
                    nc.gpsimd.dma_start(out[rs:rs + 128, :], ws)

            def make_norm(h, qt, ot, otn):
                # 1/Z on the single Z partition row, broadcast across the
                # head's 64 partitions via K=1 fp32r matmuls, staged
                # through SBUF (DVE ops keep one PSUM operand max), then
                # the normalized O^T half lands in otn -- directly for
                # head 0; via a partition-shifting DMA copy for head 1
                # (engines cannot shift partitions; the DMA can).
                # Emitted deferred -- inside the NEXT pass, after its
                # first two score matmuls -- so the reciprocal latency
                # never delays the exp stream.
                def emit(between=None):
                    rz = rzp.tile([128, 2, QH], bf16, tag="rz", name="rz")
                    rbs = rzp.tile([128, 2, QH], f32, tag="rbs", name="rbs")
                    dst = otn if h == 0 else op.tile(
                        [128, 1024], bf16, tag="ott", name="ott", bufs=2
                    )
                    dc = qt * 1024 if h == 0 else 0
                    with nc.allow_low_precision("1/Z broadcast in bf16"):
                        nc.vector.reciprocal(
                            rz[DH:DH + 1, :, :], ot[DH:DH + 1, :, :]
                        )
                    for qh in range(2):
                        rb = psp.tile([128, QH], f32, tag="pq", name="rb")
                        nc.tensor.matmul(
                            rb[0:DH, :],
                            lhsT=onesb_sb[DH:DH + 1, 0:DH],
                            rhs=rz[DH:DH + 1, qh, :],
                        )
                        # Stage 1/Z through SBUF: a DVE tensor op may read
                        # at most one PSUM operand.
                        nc.vector.tensor_copy(rbs[0:DH, qh, :], rb[0:DH, :])
                        nc.vector.tensor_mul(
                            dst[0:DH, dc + qh * QH:dc + (qh + 1) * QH],
                            ot[0:DH, qh, :], rbs[0:DH, qh, :],
                        )
                        if qh == 0 and between is not None:
                            between()
                    if h == 1:
                        nc.sync.dma_start(
                            otn[DH:128, qt * 1024:(qt + 1) * 1024],
                            dst[0:DH, :],
                        )
                return emit

            def attn_pass(b, h, qt, otn, filler, norm_prev=None, takes=None):
                hs = h * DH
                qs = b * S + qt * 1024

                def sc_mm(kc):
                    ks = b * S + kc * 128
                    sc = pss.tile([128, 2, QH], f32, tag="sc", name="sc")
                    for qh in range(2):
                        nc.tensor.matmul(
                            sc[:, qh, :],
                            lhsT=kT_sb[hs:hs + DH, ks:ks + 128],
                            rhs=qT_sb[hs:hs + DH,
                                      qs + qh * QH:qs + (qh + 1) * QH],
                        )
                    return sc

                ot = pso.tile([128, 2, QH], f32, tag="ot", name="ot")
                sc = sc_mm(0)
                for kc in range(PK):
                    attn = atp.tile([128, 2, QH], bf16, tag="attn",
                                    name="attn")
                    nc.scalar.activation(attn, sc, Exp, scale=SCALE,
                                         bias=mb_sb[:, b, kc:kc + 1])
                    if kc < PK - 1:
                        sc = sc_mm(kc + 1)
                    if kc == 0 and norm_prev is not None:
                        norm_prev()
                    # O^T[d, q] += V[k, d]^T attn[k, q]; row 64 (the ones
                    # column) accumulates Z = sum_k attn[k, q].
                    for qh in range(2):
                        nc.tensor.matmul(
                            ot[0:VW, qh, :],
                            lhsT=v2_sb[:, b, kc, h, :],
                            rhs=attn[:, qh, :],
                            start=(kc == 0), stop=(kc == PK - 1),
                            skip_group_check=True,
                        )
                    filler.take(takes.get(kc, 1) if takes else 1)
                    if kc == 0 and norm_prev is not None:
                        filler.take(4)
                return make_norm(h, qt, ot, otn)

            # ---- batch 0 prologue -------------------------------------
            # Serial prefix = only what the first attention chunks need:
            # K slice 0, V chunks 0-3, Q slices 0-1. Everything else is
            # filler inside pass 1, paced to the staged DMA arrivals.
            nc.sync.dma_start(wk_sb, wk)
            xhk = xhp.tile([128, PD, S], bf16, tag="xh", name="xhk")
            xhv = xhp.tile([128, PD, S], bf16, tag="xh", name="xhv")
            xhq = xhp.tile([128, PD, S], bf16, tag="xh", name="xhq")
            emit_load("k", 0, xhk, 0, 512, split=True)
            nc.sync.dma_start(wq_sb, wq)
            nc.sync.dma_start(wv_sb, wv)
            nc.sync.dma_start(wo_sb, wo)
            nc.sync.dma_start(mb_sb, mb)
            # Tiny warm-up ops: let DVE/ACT observe the mask DMA early and
            # pull the ~2.7us exp table load off the critical path.
            scratch = wpool.tile([1, 2], f32)
            nc.vector.tensor_copy(scratch, mb_sb[0:1, 0, 0:2])
            scratch2 = wpool.tile([1, 2], f32)
            nc.scalar.activation(scratch2, mb_sb[0:1, 0, 0:2], Exp)
            emit_load("v", 0, xhv, 0, 512, split=True)
            emit_load("q", 0, xhq, 0, 512, split=True)
            emit_load("q", 0, xhq, 512, 512, split=True)
            emit_load("k", 0, xhk, 512, 512, split=True)
            emit_load("v", 0, xhv, 512, 512, split=True)
            emit_load("k", 0, xhk, 1024, 512, split=True)
            emit_load("v", 0, xhv, 1024, 512, split=True)
            emit_load("k", 0, xhk, 1536, 512, split=True)
            emit_load("v", 0, xhv, 1536, 512, split=True)
            emit_load("q", 0, xhq, 1024, 512, split=True)
            emit_load("q", 0, xhq, 1536, 512, split=True)

            run_gen(emit_group("k", 0, xhk, 0))
            run_gen(emit_group("v", 0, xhv, 0))
            run_gen(emit_group("q", 0, xhq, 0))
            run_gen(emit_group("q", 0, xhq, 1))

            def gen_b0_rest():
                yield from emit_group("k", 0, xhk, 1)
                yield from emit_group("v", 0, xhv, 1)
                yield from emit_group("k", 0, xhk, 2)
                yield from emit_group("v", 0, xhv, 2)
                yield from emit_group("k", 0, xhk, 3)
                yield from emit_group("v", 0, xhv, 3)
                yield from emit_group("q", 0, xhq, 2)
                yield from emit_group("q", 0, xhq, 3)

            # ---- attention, software-pipelined ------------------------
            fil = _Filler(gen_b0_rest(), gen_proj(1))
            otn0 = op.tile([128, S], bf16, tag="otn", name="otn0")
            otn1 = op.tile([128, S], bf16, tag="otn", name="otn1")
            # pass 1: K/V/Q tails land mid-pass; pace the filler to the
            # staged DMA arrivals.
            nrm = attn_pass(0, 0, 0, otn0, fil,
                            takes={0: 8, 1: 8, 2: 0, 3: 0, 4: 8, 5: 0,
                                   6: 8, 7: 0, 8: 0, 9: 8, 10: 0, 11: 8,
                                   12: 0, 13: 8, 14: 0, 15: 8})
            nrm = attn_pass(0, 0, 1, otn0, fil, norm_prev=nrm,
                            takes={kc: 2 if kc % 2 == 0 else 1
                                   for kc in range(PK)})
            nrm = attn_pass(0, 1, 0, otn0, fil, norm_prev=nrm,
                            takes={kc: 2 if kc % 2 == 0 else 1
                                   for kc in range(PK)})
            nrm = attn_pass(0, 1, 1, otn0, fil, norm_prev=nrm,
                            takes={kc: 2 for kc in range(PK)})

            # Batch 1 runs head 1 first so the LAST pass is head 0, whose
            # normalization writes otn directly -- no partition-shift DMA
            # on the tail critical path.
            fil.add(gen_outproj(0, otn0, 0, S // 128))
            nrm = attn_pass(1, 1, 0, otn1, fil, norm_prev=nrm,
                            takes={kc: 2 if kc < 8 else 1
                                   for kc in range(PK)})
            nrm = attn_pass(1, 1, 1, otn1, fil, norm_prev=nrm)
            nrm = attn_pass(1, 0, 0, otn1, fil, norm_prev=nrm)
            fil.add(gen_outproj(1, otn1, 0, 8))
            nrm = attn_pass(1, 0, 1, otn1, fil, norm_prev=nrm)

            # ---- tail: final normalization + remaining output proj ----
            # The first half of the remaining output projection depends
            # only on the qh=0 normalization half; interleave it so it
            # overlaps the qh=1 chain.
            nrm(between=lambda: run_gen(gen_outproj(1, otn1, 8, 12,
                                                    tail=True)))
            fil.add(gen_outproj(1, otn1, 12, S // 128, tail=True))
            fil.take(1 << 20)

    nc.compile()
    _NC_CACHE["nc"] = nc
    return nc


def _prep_inputs(queries, keys, values, masks, Wq, Wk, Wv, Wo):
    """Host-side sharding/layout prep. Returns per-core input maps."""
    def t_bf16(x):  # [B, S, D] -> [D, B*S] bf16, contiguous
        return np.ascontiguousarray(
            np.asarray(x, dtype=np.float32).reshape(BS, D).astype(BF16).T
        )

    xqT, xkT, xvT = t_bf16(queries), t_bf16(keys), t_bf16(values)

    m01 = (np.asarray(masks) != 0).astype(np.float32)          # [B, S]
    mb = np.ascontiguousarray(
        np.where(m01.reshape(B, PK, 128) != 0, 0.0, -30000.0)
        .transpose(2, 0, 1).astype(np.float32)
    )

    def w_prep(W, c):  # [D, D] -> [128, PD, MH] bf16 slice for core c
        Wc = np.asarray(W, dtype=np.float32)[:, c * MH:(c + 1) * MH]
        return np.ascontiguousarray(
            Wc.astype(BF16).reshape(PD, 128, MH).transpose(1, 0, 2)
        )

    Wo_f = np.asarray(Wo, dtype=np.float32)
    in_maps = []
    for c in range(NCORES):
        in_maps.append({
            "xqT": xqT, "xkT": xkT, "xvT": xvT,
            "wq": w_prep(Wq, c), "wk": w_prep(Wk, c), "wv": w_prep(Wv, c),
            "wo": np.ascontiguousarray(
                Wo_f[c * MH:(c + 1) * MH, :].astype(BF16)
            ),
            "mb": mb,
        })
    return in_maps


def run(inputs, trace=False, trace_cores=None):
    """Run on 8 NeuronCores; returns (output [B,S,D] f32, BassKernelResults)."""
    from concourse.bass_utils import run_bass_kernel_spmd

    nc = _build_nc()
    in_maps = _prep_inputs(**inputs)
    res = run_bass_kernel_spmd(
        nc, in_maps, core_ids=list(range(NCORES)),
        trace=trace, trace_cores=trace_cores,
    )
    acc = res.results[0]["out"].astype(np.float32, copy=True)
    for r in res.results[1:]:
        acc += r["out"]
    return acc.reshape(B, S, D), res


def kernel(**inputs) -> np.ndarray:
    out, _ = run(inputs)
    return out
